# revision 14
# baseline (speedup 1.0000x reference)
"""BezierAlign Trainium2 kernel.

Full inputs -> full output. Shards the R=256 ROIs across 8 NeuronCores (32
ROIs/core); the feature map is replicated to every core in a "window block"
layout: block(n,y,x) holds the 4-wide x 2-tall fp16 pixel window
[f(y,x..x+3) x f(y..y+1)] = 8C values = 4KB, so ONE indirect-DMA descriptor
fetches the footprint of BOTH x-samples of a bin row (max bin width 2.5 px
=> x_low spread <= 2). Indirect gathers are descriptor-rate-bound (~8.6 ns
per descriptor), so descriptor count is what matters.

Partition layout packs the TWO y-sample rows into the matmul contraction:
p = iy*64 + w (iy = y-sample row, w = output column). A tile is one output
row (64 bins). One gather per tile fetches 128 rows = both y-samples of all
64 bins; one matmul per (window-block, C-half) contracts both y-samples
via a stacked double-diagonal rhs [128, 64], so the PE streams only 64
columns per matmul (16 col/bin total vs 32 for the naive scheme) and the
diag-build DVE work is halved.

Per-core device program:
  1. Evaluate the 4 cubic Bezier curves per ROI on 32 partitions (roi-major),
     fold the +-0.25*bin sample offsets and the -0.5 align shift into shifted
     endpoint curves, PE-transpose them (x-curves duplicated across halves,
     y-curves iy-stacked) to the p = (iy, w) layout.
  2. Per ROI, compute sample coords / validity / per-window weights W8
     (4 x-positions x 2 y-rows, with dx-selection masks merging the two
     x-samples) / gather offsets, all on (iy, w) partitions, f32 DVE.
  3. Per 64-bin tile: 1 indirect gather ([128, 8C] fp16), 8 double-diag
     builds, 16 fp16 matmuls accumulating [C-half, 64 bins] in 2 PSUM
     tiles; copy to SBUF, DMA out.
"""

import numpy as np

# problem shapes (hardcoded per contract)
N, C, H, W = 2, 256, 160, 160
R = 256
OUT_H, OUT_W = 16, 64
SCALE = 0.25
NCORES = 8
K = R // NCORES          # 32 rois per core
NT = OUT_H               # 16 tiles of 64 bins (one output row) per roi
HW = H * W
NB = 8                   # blocks per window: 4 dx * 2 y

_CACHE = {}


def _host_constants():
    f32 = np.float32
    u = (np.arange(OUT_W, dtype=f32) / f32(OUT_W)).astype(f32)
    mt = (f32(1.0) - u).astype(f32)
    basis = np.stack([mt**3, 3 * u * mt**2, 3 * u**2 * mt, u**3]).astype(f32)  # [4,64]
    basis32 = np.broadcast_to(basis.reshape(1, 4 * OUT_W), (K, 4 * OUT_W)).copy()
    t = np.arange(NT, dtype=f32) / f32(NT)
    v16 = np.broadcast_to(t[None, :], (128, NT)).copy()      # [128,16] row v
    return basis32, v16


def _build_feat8(x):
    """x [N, C, H, W] f32 -> fp16 [N*H*W, 8C]; block(n,y,x) =
    [f(y,x), f(y+1,x), f(y,x+1), f(y+1,x+1), ..., f(y,x+3), f(y+1,x+3)]
    with out-of-image parts zeroed."""
    f = np.ascontiguousarray(x.transpose(0, 2, 3, 1)).astype(np.float16)  # [N,H,W,C]
    fy = np.zeros_like(f)
    fy[:, :-1] = f[:, 1:]
    a = np.concatenate([f, fy], axis=-1)                  # [N,H,W,2C] y-pair
    parts = [a]
    for dx in (1, 2, 3):
        ax = np.zeros_like(a)
        ax[:, :, :-dx] = a[:, :, dx:]
        parts.append(ax)
    feat8 = np.concatenate(parts, axis=-1)                # [N,H,W,8C]
    return np.ascontiguousarray(feat8.reshape(N * HW, NB * C))


def _build_nc(nrep=1):
    from contextlib import ExitStack
    import concourse.bacc as bacc
    import concourse.bass as bass
    import concourse.tile as tile
    from concourse import mybir
    from concourse.masks import make_identity

    f32 = mybir.dt.float32
    f16 = mybir.dt.float16
    i32 = mybir.dt.int32
    Alu = mybir.AluOpType

    nc = bacc.Bacc(None, target_bir_lowering=False)

    feat8 = nc.dram_tensor("feat8", [N * HW, NB * C], f16, kind="ExternalInput")
    rois = nc.dram_tensor("rois", [K, 17], f32, kind="ExternalInput")
    basis = nc.dram_tensor("basis", [K, 4 * OUT_W], f32, kind="ExternalInput")
    v16c = nc.dram_tensor("v16c", [128, NT], f32, kind="ExternalInput")
    out = nc.dram_tensor("out", [K, C, OUT_H, OUT_W], f32, kind="ExternalOutput")
    # [K, C, 16, 64] -> (k, p, h, t, w): c = h*128 + p
    out_v = out.rearrange("k (h p) t w -> k p h t w", h=2)

    with tile.TileContext(nc) as tc, ExitStack() as ctx:
        singles = ctx.enter_context(tc.tile_pool(name="singles", bufs=1))
        scratch = ctx.enter_context(tc.tile_pool(name="scratch", bufs=2))
        tabs = ctx.enter_context(tc.tile_pool(name="tabs", bufs=3))
        gpool = ctx.enter_context(tc.tile_pool(name="gpool", bufs=8))
        dpool = ctx.enter_context(tc.tile_pool(name="dpool", bufs=24))
        spool = ctx.enter_context(tc.tile_pool(name="spool", bufs=6))
        pp_t = ctx.enter_context(tc.tile_pool(name="pp_t", bufs=1, space="PSUM"))
        pp_mm = ctx.enter_context(tc.tile_pool(name="pp_mm", bufs=3, space="PSUM"))

        ident = singles.tile([128, 128], f32)
        make_identity(nc, ident[:])
        ident_h = singles.tile([128, 128], f16)
        nc.vector.tensor_copy(out=ident_h[:], in_=ident[:])
        # ID2 [128, 64] fp16: stacked pair of 64-identities (rows 0-63 and
        # 64-127 both diag on columns 0-63)
        ID2 = singles.tile([128, 64], f16)
        nc.vector.tensor_copy(out=ID2[0:64, :], in_=ident_h[0:64, 0:64])
        nc.vector.tensor_copy(out=ID2[64:128, :], in_=ident_h[64:128, 64:128])
        v16_t = singles.tile([128, NT], f32)
        nc.sync.dma_start(out=v16_t[:], in_=v16c[:])
        r_t = singles.tile([K, 17], f32)
        nc.sync.dma_start(out=r_t[:], in_=rois[:])
        b_t = singles.tile([K, 4, OUT_W], f32)
        nc.sync.dma_start(out=b_t[:], in_=basis[:].rearrange("k (a u) -> k a u", a=4))

        # control points: px = rois[:, 1::2]*0.25, py = rois[:, 2::2]*0.25
        px = scratch.tile([K, 8], f32, tag="px")
        py = scratch.tile([K, 8], f32, tag="py")
        r_ap = r_t[:]
        px_src = bass.AP(tensor=r_ap.tensor, offset=r_ap.offset + 1, ap=[list(r_ap.ap[0]), [2, 8]])
        py_src = bass.AP(tensor=r_ap.tensor, offset=r_ap.offset + 2, ap=[list(r_ap.ap[0]), [2, 8]])
        nc.vector.tensor_scalar(out=px[:], in0=px_src, scalar1=SCALE, scalar2=None, op0=Alu.mult)
        nc.vector.tensor_scalar(out=py[:], in0=py_src, scalar1=SCALE, scalar2=None, op0=Alu.mult)

        # curves [K, 64]: cv = sum_a B[a] * p[a(+4)]
        def bezier(dst, ptile, o):
            acc = scratch.tile([K, OUT_W], f32, tag="bzacc")
            tmp = scratch.tile([K, OUT_W], f32, tag="bztmp")
            nc.vector.tensor_scalar(out=acc[:], in0=b_t[:, 0, :], scalar1=ptile[:, o:o+1],
                                    scalar2=None, op0=Alu.mult)
            for a in (1, 2, 3):
                nc.vector.tensor_scalar(out=tmp[:], in0=b_t[:, a, :], scalar1=ptile[:, o+a:o+a+1],
                                        scalar2=None, op0=Alu.mult)
                nc.vector.tensor_tensor(out=dst[:] if a == 3 else acc[:],
                                        in0=acc[:], in1=tmp[:], op=Alu.add)

        x0 = scratch.tile([K, OUT_W], f32, tag="x0"); bezier(x0, px, 0)
        x1 = scratch.tile([K, OUT_W], f32, tag="x1"); bezier(x1, px, 4)
        y0 = scratch.tile([K, OUT_W], f32, tag="y0"); bezier(y0, py, 0)
        y1 = scratch.tile([K, OUT_W], f32, tag="y1"); bezier(y1, py, 4)

        # roi_w/h -> bwq = roi_w*0.25/64, bhq = roi_h*0.25/16  [K,1]
        def quarter_bin(ptile, scale_imm, tag):
            d1 = scratch.tile([K, 1], f32, tag=tag + "d1")
            d2 = scratch.tile([K, 1], f32, tag=tag + "d2")
            dn = scratch.tile([K, 1], f32, tag=tag + "dn")
            q = scratch.tile([K, 1], f32, tag=tag)
            nc.vector.tensor_tensor(out=d1[:], in0=ptile[:, 0:1], in1=ptile[:, 3:4], op=Alu.subtract)
            nc.vector.tensor_scalar(out=dn[:], in0=d1[:], scalar1=-1.0, scalar2=None, op0=Alu.mult)
            nc.vector.tensor_tensor(out=d1[:], in0=d1[:], in1=dn[:], op=Alu.max)
            nc.vector.tensor_tensor(out=d2[:], in0=ptile[:, 4:5], in1=ptile[:, 7:8], op=Alu.subtract)
            nc.vector.tensor_scalar(out=dn[:], in0=d2[:], scalar1=-1.0, scalar2=None, op0=Alu.mult)
            nc.vector.tensor_tensor(out=d2[:], in0=d2[:], in1=dn[:], op=Alu.max)
            nc.vector.tensor_tensor(out=d1[:], in0=d1[:], in1=d2[:], op=Alu.max)
            nc.vector.tensor_scalar(out=q[:], in0=d1[:], scalar1=scale_imm, scalar2=None, op0=Alu.mult)
            return q

        bwq = quarter_bin(px, 0.25 / OUT_W, "bwq")
        bhq = quarter_bin(py, 0.25 / OUT_H, "bhq")

        # 8 shifted curves [K, 64]: order xm0 xm1 xp0 xp1 ym0 ym1 yp0 yp1
        curves = scratch.tile([K, 9, OUT_W], f32, tag="curves")
        spec = [(x0, bwq, Alu.subtract, 0), (x1, bwq, Alu.subtract, 1),
                (x0, bwq, Alu.add, 2), (x1, bwq, Alu.add, 3),
                (y0, bhq, Alu.subtract, 4), (y1, bhq, Alu.subtract, 5),
                (y0, bhq, Alu.add, 6), (y1, bhq, Alu.add, 7)]
        for cv, qq, op, idx in spec:
            nc.vector.tensor_scalar(out=curves[:, idx, :], in0=cv[:], scalar1=qq[:, 0:1],
                                    scalar2=0.5, op0=op, op1=Alu.subtract)
        # base = batch * HW broadcast along 64
        base_c = scratch.tile([K, 1], f32, tag="base_c")
        nc.vector.tensor_scalar(out=base_c[:], in0=r_t[:, 0:1], scalar1=float(HW),
                                scalar2=None, op0=Alu.mult)
        bc_ap = base_c[:]
        nc.vector.tensor_scalar(
            out=curves[:, 8, :],
            in0=bass.AP(tensor=bc_ap.tensor, offset=bc_ap.offset, ap=[list(bc_ap.ap[0]), [0, OUT_W]]),
            scalar1=0.0, scalar2=None, op0=Alu.add)

        # transpose to TT [128, 7, K], p = iy*64 + w:
        #  slots 0-3: x endpoint curves xm0 xp0 xm1 xp1, duplicated across
        #             iy halves (x is iy-independent)
        #  slot 4: Y0 = [ym0 | yp0] iy-stacked; slot 5: Y1 = [ym1 | yp1]
        #  slot 6: base, duplicated
        TT = singles.tile([128, 7, K], f32)
        tt_spec = [(0, (0, 0)), (1, (1, 1)), (2, (2, 2)), (3, (3, 3)),
                   (4, (4, 6)), (5, (5, 7)), (6, (8, 8))]
        for slot, (qlo, qhi) in tt_spec:
            ps = pp_t.tile([128, K], f32, tag="tps", space="PSUM")
            cdup = scratch.tile([K, 128], f32, tag="cdup")
            nc.vector.tensor_copy(out=cdup[:, 0:64], in_=curves[:, qlo, :])
            nc.vector.tensor_copy(out=cdup[:, 64:128], in_=curves[:, qhi, :])
            nc.tensor.transpose(out=ps[:], in_=cdup[:], identity=ident[:K, :K])
            nc.vector.tensor_copy(out=TT[:, slot, :], in_=ps[:])

        def ttcol(q, r):
            return TT[:, q, r:r+1]

        IX, T16 = 2, NT

        def main_work():
         for r in range(K):
            # deltas [128,1]: x-lerp slopes per ix, y-lerp slope (iy-stacked)
            dxm = tabs.tile([128, 1], f32, tag="dxm")
            dxp = tabs.tile([128, 1], f32, tag="dxp")
            dyy = tabs.tile([128, 1], f32, tag="dyy")
            nc.vector.tensor_tensor(out=dxm[:], in0=ttcol(1, r), in1=ttcol(0, r), op=Alu.subtract)
            nc.vector.tensor_tensor(out=dxp[:], in0=ttcol(3, r), in1=ttcol(2, r), op=Alu.subtract)
            nc.vector.tensor_tensor(out=dyy[:], in0=ttcol(5, r), in1=ttcol(4, r), op=Alu.subtract)

            # XX [128, 2(ix), 16(t)]; YY [128, 16(t)] (iy via partition half)
            XX = tabs.tile([128, IX, T16], f32, tag="XX")
            YY = tabs.tile([128, T16], f32, tag="YY")
            nc.vector.tensor_scalar(out=XX[:, 0, :], in0=v16_t[:], scalar1=dxm[:, 0:1],
                                    scalar2=ttcol(0, r), op0=Alu.mult, op1=Alu.add)
            nc.vector.tensor_scalar(out=XX[:, 1, :], in0=v16_t[:], scalar1=dxp[:, 0:1],
                                    scalar2=ttcol(2, r), op0=Alu.mult, op1=Alu.add)
            nc.vector.tensor_scalar(out=YY[:], in0=v16_t[:], scalar1=dyy[:, 0:1],
                                    scalar2=ttcol(4, r), op0=Alu.mult, op1=Alu.add)

            # coord pipe
            def pipe(P, F, limit, tagp):
                vv = tabs.tile([128, F], f32, tag=tagp + "v")
                v2 = tabs.tile([128, F], f32, tag=tagp + "v2")
                xx = tabs.tile([128, F], f32, tag=tagp + "x")
                xi = tabs.tile([128, F], i32, tag=tagp + "i")
                xf = tabs.tile([128, F], f32, tag=tagp + "f")
                xfc = tabs.tile([128, F], f32, tag=tagp + "fc")
                lo = tabs.tile([128, F], f32, tag=tagp + "lo")
                mm = tabs.tile([128, F], f32, tag=tagp + "m")
                lx = tabs.tile([128, F], f32, tag=tagp + "l")
                hx = tabs.tile([128, F], f32, tag=tagp + "h")
                nc.vector.tensor_scalar(out=vv[:], in0=P, scalar1=-1.0, scalar2=None, op0=Alu.is_gt)
                nc.vector.tensor_scalar(out=v2[:], in0=P, scalar1=float(limit), scalar2=None, op0=Alu.is_lt)
                nc.vector.tensor_tensor(out=vv[:], in0=vv[:], in1=v2[:], op=Alu.mult)
                nc.vector.tensor_scalar(out=xx[:], in0=P, scalar1=0.0, scalar2=None, op0=Alu.max)
                nc.vector.tensor_scalar(out=xi[:], in0=xx[:], scalar1=0.5, scalar2=None, op0=Alu.subtract)
                nc.vector.tensor_copy(out=xf[:], in_=xi[:])
                nc.vector.tensor_scalar(out=xfc[:], in0=xf[:], scalar1=float(limit - 1),
                                        scalar2=None, op0=Alu.min)
                nc.vector.tensor_tensor(out=lo[:], in0=xx[:], in1=xfc[:], op=Alu.subtract)
                nc.vector.tensor_scalar(out=mm[:], in0=xfc[:], scalar1=float(limit - 1),
                                        scalar2=None, op0=Alu.is_lt)
                nc.vector.tensor_tensor(out=lx[:], in0=lo[:], in1=mm[:], op=Alu.mult)
                nc.vector.tensor_scalar(out=hx[:], in0=lx[:], scalar1=-1.0, scalar2=1.0,
                                        op0=Alu.mult, op1=Alu.add)
                return vv, xfc, lx, hx

            vx, xfc, lx, hx = pipe(XX[:].rearrange("p a t -> p (a t)"), IX * T16, W, "px")
            vy, yfc, ly, hy = pipe(YY[:], T16, H, "py")

            # per-sample x validity fold: hxv = hx*vx, lxv = lx*vx [128, 32]
            hxv = tabs.tile([128, IX * T16], f32, tag="hxv")
            lxv = tabs.tile([128, IX * T16], f32, tag="lxv")
            nc.vector.tensor_tensor(out=hxv[:], in0=hx[:], in1=vx[:], op=Alu.mult)
            nc.vector.tensor_tensor(out=lxv[:], in0=lx[:], in1=vx[:], op=Alu.mult)

            # window-position indicators: d1 = xfc1 - xfc0 in {0,1,2} [128,16]
            d1 = tabs.tile([128, T16], f32, tag="d1")
            nc.vector.tensor_tensor(out=d1[:], in0=xfc[:, T16:2*T16], in1=xfc[:, 0:T16],
                                    op=Alu.subtract)
            i0 = tabs.tile([128, T16], f32, tag="i0")
            i1 = tabs.tile([128, T16], f32, tag="i1")
            i2 = tabs.tile([128, T16], f32, tag="i2")
            nc.vector.tensor_scalar(out=i0[:], in0=d1[:], scalar1=0.5, scalar2=None, op0=Alu.is_lt)
            nc.vector.tensor_scalar(out=i2[:], in0=d1[:], scalar1=1.5, scalar2=None, op0=Alu.is_gt)
            nc.vector.tensor_tensor(out=i1[:], in0=i0[:], in1=i2[:], op=Alu.add)
            nc.vector.tensor_scalar(out=i1[:], in0=i1[:], scalar1=-1.0, scalar2=1.0,
                                    op0=Alu.mult, op1=Alu.add)

            # x-weight planes WXT [128, 4(dx), 16(t)]
            # wx0 = hxv0 + i0*hxv1 ; wx1 = lxv0 + i0*lxv1 + i1*hxv1
            # wx2 = i1*lxv1 + i2*hxv1 ; wx3 = i2*lxv1
            WXT = tabs.tile([128, 4, T16], f32, tag="WXT")
            tmpa = tabs.tile([128, T16], f32, tag="tmpa")
            tmpb = tabs.tile([128, T16], f32, tag="tmpb")
            h1 = hxv[:, T16:2*T16]
            l1 = lxv[:, T16:2*T16]
            nc.vector.tensor_tensor(out=tmpa[:], in0=i0[:], in1=h1, op=Alu.mult)
            nc.vector.tensor_tensor(out=WXT[:, 0, :], in0=hxv[:, 0:T16], in1=tmpa[:], op=Alu.add)
            nc.vector.tensor_tensor(out=tmpa[:], in0=i0[:], in1=l1, op=Alu.mult)
            nc.vector.tensor_tensor(out=tmpb[:], in0=i1[:], in1=h1, op=Alu.mult)
            nc.vector.tensor_tensor(out=tmpa[:], in0=tmpa[:], in1=tmpb[:], op=Alu.add)
            nc.vector.tensor_tensor(out=WXT[:, 1, :], in0=lxv[:, 0:T16], in1=tmpa[:], op=Alu.add)
            nc.vector.tensor_tensor(out=tmpa[:], in0=i1[:], in1=l1, op=Alu.mult)
            nc.vector.tensor_tensor(out=tmpb[:], in0=i2[:], in1=h1, op=Alu.mult)
            nc.vector.tensor_tensor(out=WXT[:, 2, :], in0=tmpa[:], in1=tmpb[:], op=Alu.add)
            nc.vector.tensor_tensor(out=WXT[:, 3, :], in0=i2[:], in1=l1, op=Alu.mult)

            # offsets: o = (yfc*W + base) + xfc0  [128, 16] -> int32
            yw = tabs.tile([128, T16], f32, tag="yw")
            nc.vector.tensor_scalar(out=yw[:], in0=yfc[:], scalar1=float(W),
                                    scalar2=ttcol(6, r), op0=Alu.mult, op1=Alu.add)
            of = tabs.tile([128, T16], f32, tag="of")
            nc.vector.tensor_tensor(out=of[:], in0=yw[:], in1=xfc[:, 0:T16], op=Alu.add)
            O = tabs.tile([128, T16], i32, tag="O")
            nc.vector.tensor_copy(out=O[:], in_=of[:])

            # y weights folded with valid & 1/4: hyq = hy*vy*0.25, lyq likewise
            q0 = tabs.tile([128, T16], f32, tag="q0")
            nc.vector.tensor_scalar(out=q0[:], in0=vy[:], scalar1=0.25, scalar2=None, op0=Alu.mult)
            hyq = tabs.tile([128, T16], f32, tag="hyq")
            lyq = tabs.tile([128, T16], f32, tag="lyq")
            nc.vector.tensor_tensor(out=hyq[:], in0=hy[:], in1=q0[:], op=Alu.mult)
            nc.vector.tensor_tensor(out=lyq[:], in0=ly[:], in1=q0[:], op=Alu.mult)

            # W8 [128, 8 (2dx+ylh), 16 (t)]: wx[dx] x (hyq, lyq)
            W8 = tabs.tile([128, NB, T16], f32, tag="W8")
            for dx in range(4):
                nc.vector.tensor_tensor(out=W8[:, 2*dx, :], in0=hyq[:], in1=WXT[:, dx, :], op=Alu.mult)
                nc.vector.tensor_tensor(out=W8[:, 2*dx+1, :], in0=lyq[:], in1=WXT[:, dx, :], op=Alu.mult)

            # per tile (one output row, 64 bins): gather + combine into
            # [C-half, 64] PSUM via double-diag matmuls
            st = None
            for t in range(T16):
                g = gpool.tile([128, NB * C], f16, tag="g")
                nc.gpsimd.indirect_dma_start(
                    out=g[:], out_offset=None, in_=feat8[:],
                    in_offset=bass.IndirectOffsetOnAxis(ap=O[:, t:t+1], axis=0))
                # all 8 double-diags for this tile in ONE DVE op:
                # DG8[p, blk, b] = ID2[p, b] * W8[p, blk, t]
                DG8 = dpool.tile([128, NB, 64], f16, tag="DG8")
                id_ap = ID2[:]
                id_b = bass.AP(tensor=id_ap.tensor, offset=id_ap.offset,
                               ap=[list(id_ap.ap[0]), [0, NB], [1, 64]])
                w8_ap = W8[:]
                w8_b = bass.AP(tensor=w8_ap.tensor, offset=w8_ap.offset + t,
                               ap=[list(w8_ap.ap[0]), [T16, NB], [0, 64]])
                nc.vector.tensor_tensor(out=DG8[:], in0=id_b, in1=w8_b, op=Alu.mult)
                psA = pp_mm.tile([128, 64], f32, tag="psA", space="PSUM")
                psB = pp_mm.tile([128, 64], f32, tag="psB", space="PSUM")
                for blk in range(NB):
                    nc.tensor.matmul(psA[:], lhsT=g[:, blk*C:blk*C+128],
                                     rhs=DG8[:, blk, :],
                                     start=(blk == 0), stop=(blk == NB - 1))
                    nc.tensor.matmul(psB[:], lhsT=g[:, blk*C+128:blk*C+256],
                                     rhs=DG8[:, blk, :],
                                     start=(blk == 0), stop=(blk == NB - 1))
                # stage 2 tiles per out-DMA (512B runs per partition)
                if t % 2 == 0:
                    st = spool.tile([128, 2, 2, 64], f32, tag="st")
                nc.scalar.copy(st[:, 0, t % 2, :], psA[:])
                nc.scalar.copy(st[:, 1, t % 2, :], psB[:])
                if t % 2 == 1:
                    nc.sync.dma_start(out=out_v[r, :, :, t-1:t+1, :], in_=st[:])

        if nrep > 1:
            with tc.For_i(0, nrep, 1):
                main_work()
        else:
            main_work()

    nc.finalize()
    return nc


def _get_nc():
    if "nc" not in _CACHE:
        _CACHE["nc"] = _build_nc()
    return _CACHE["nc"]


def run_sharded(input, rois, **spmd_kwargs):
    """Run on 8 cores; returns (full_output, BassKernelResults)."""
    from concourse.bass_utils import run_bass_kernel_spmd

    x = np.ascontiguousarray(np.asarray(input, dtype=np.float32))
    rr = np.ascontiguousarray(np.asarray(rois, dtype=np.float32))
    feat8 = _build_feat8(x)
    basis32, v16 = _host_constants()

    in_maps = []
    for c in range(NCORES):
        in_maps.append({
            "feat8": feat8,
            "rois": np.ascontiguousarray(rr[c*K:(c+1)*K]),
            "basis": basis32,
            "v16c": v16,
        })
    nc = _get_nc()
    res = run_bass_kernel_spmd(nc, in_maps, core_ids=list(range(NCORES)), **spmd_kwargs)
    outp = np.concatenate([res.results[c]["out"] for c in range(NCORES)], axis=0)
    return outp, res


def kernel(input, rois):
    out, _ = run_sharded(input, rois)
    return out


# revision 15
# speedup vs baseline: 1.0936x; 1.0936x over previous
"""BezierAlign Trainium2 kernel.

Full inputs -> full output. Shards the R=256 ROIs across 8 NeuronCores (32
ROIs/core); the feature map is replicated to every core in a "window block"
layout: block(n,y,x) holds the 4-wide x 2-tall fp16 pixel window
[f(y,x..x+3) x f(y..y+1)] = 8C values = 4KB, so ONE indirect-DMA descriptor
fetches the footprint of BOTH x-samples of a bin row (max bin width 2.5 px
=> x_low spread <= 2). Indirect gathers are descriptor-rate-bound (~8.6 ns
per descriptor), so descriptor count is what matters.

Partition layout packs the TWO y-sample rows into the matmul contraction:
p = iy*64 + w (iy = y-sample row, w = output column). A tile is one output
row (64 bins). One gather per tile fetches 128 rows = both y-samples of all
64 bins; one matmul per (window-block, C-half) contracts both y-samples
via a stacked double-diagonal rhs [128, 64], so the PE streams only 64
columns per matmul (16 col/bin total vs 32 for the naive scheme) and the
diag-build DVE work is halved.

Per-core device program:
  1. Evaluate the 4 cubic Bezier curves per ROI on 32 partitions (roi-major),
     fold the +-0.25*bin sample offsets and the -0.5 align shift into shifted
     endpoint curves, PE-transpose them (x-curves duplicated across halves,
     y-curves iy-stacked) to the p = (iy, w) layout.
  2. Per ROI, compute sample coords / validity / per-window weights W8
     (4 x-positions x 2 y-rows, with dx-selection masks merging the two
     x-samples) / gather offsets, all on (iy, w) partitions, f32 DVE.
  3. Per 64-bin tile: 1 indirect gather ([128, 8C] fp16), 8 double-diag
     builds, 16 fp16 matmuls accumulating [C-half, 64 bins] in 2 PSUM
     tiles; copy to SBUF, DMA out.
"""

import numpy as np

# problem shapes (hardcoded per contract)
N, C, H, W = 2, 256, 160, 160
R = 256
OUT_H, OUT_W = 16, 64
SCALE = 0.25
NCORES = 8
K = R // NCORES          # 32 rois per core
NT = OUT_H               # 16 tiles of 64 bins (one output row) per roi
HW = H * W
NB = 8                   # blocks per window: 4 dx * 2 y

_CACHE = {}


def _host_constants():
    f32 = np.float32
    u = (np.arange(OUT_W, dtype=f32) / f32(OUT_W)).astype(f32)
    mt = (f32(1.0) - u).astype(f32)
    basis = np.stack([mt**3, 3 * u * mt**2, 3 * u**2 * mt, u**3]).astype(f32)  # [4,64]
    basis32 = np.broadcast_to(basis.reshape(1, 4 * OUT_W), (K, 4 * OUT_W)).copy()
    t = np.arange(NT, dtype=f32) / f32(NT)
    v16 = np.broadcast_to(t[None, :], (128, NT)).copy()      # [128,16] row v
    return basis32, v16


def _build_feat8(x):
    """x [N, C, H, W] f32 -> fp16 [N*H*W, 8C]; block(n,y,x) =
    [f(y,x), f(y+1,x), f(y,x+1), f(y+1,x+1), ..., f(y,x+3), f(y+1,x+3)]
    with out-of-image parts zeroed."""
    f = np.ascontiguousarray(x.transpose(0, 2, 3, 1)).astype(np.float16)  # [N,H,W,C]
    fy = np.zeros_like(f)
    fy[:, :-1] = f[:, 1:]
    a = np.concatenate([f, fy], axis=-1)                  # [N,H,W,2C] y-pair
    parts = [a]
    for dx in (1, 2, 3):
        ax = np.zeros_like(a)
        ax[:, :, :-dx] = a[:, :, dx:]
        parts.append(ax)
    feat8 = np.concatenate(parts, axis=-1)                # [N,H,W,8C]
    return np.ascontiguousarray(feat8.reshape(N * HW, NB * C))


def _build_nc(nrep=1):
    from contextlib import ExitStack
    import concourse.bacc as bacc
    import concourse.bass as bass
    import concourse.tile as tile
    from concourse import mybir
    from concourse.masks import make_identity

    f32 = mybir.dt.float32
    f16 = mybir.dt.float16
    i32 = mybir.dt.int32
    Alu = mybir.AluOpType

    nc = bacc.Bacc(None, target_bir_lowering=False)

    feat8 = nc.dram_tensor("feat8", [N * HW, NB * C], f16, kind="ExternalInput")
    rois = nc.dram_tensor("rois", [K, 17], f32, kind="ExternalInput")
    basis = nc.dram_tensor("basis", [K, 4 * OUT_W], f32, kind="ExternalInput")
    v16c = nc.dram_tensor("v16c", [128, NT], f32, kind="ExternalInput")
    out = nc.dram_tensor("out", [K, C, OUT_H, OUT_W], f32, kind="ExternalOutput")
    # [K, C, 16, 64] -> (k, p, h, t, w): c = h*128 + p
    out_v = out.rearrange("k (h p) t w -> k p h t w", h=2)

    with tile.TileContext(nc) as tc, ExitStack() as ctx:
        singles = ctx.enter_context(tc.tile_pool(name="singles", bufs=1))
        scratch = ctx.enter_context(tc.tile_pool(name="scratch", bufs=2))
        tabs = ctx.enter_context(tc.tile_pool(name="tabs", bufs=3))
        gpool = ctx.enter_context(tc.tile_pool(name="gpool", bufs=8))
        dpool = ctx.enter_context(tc.tile_pool(name="dpool", bufs=24))
        spool = ctx.enter_context(tc.tile_pool(name="spool", bufs=6))
        pp_t = ctx.enter_context(tc.tile_pool(name="pp_t", bufs=1, space="PSUM"))
        pp_mm = ctx.enter_context(tc.tile_pool(name="pp_mm", bufs=3, space="PSUM"))

        ident = singles.tile([128, 128], f32)
        make_identity(nc, ident[:])
        ident_h = singles.tile([128, 128], f16)
        nc.vector.tensor_copy(out=ident_h[:], in_=ident[:])
        # ID2 [128, 64] fp16: stacked pair of 64-identities (rows 0-63 and
        # 64-127 both diag on columns 0-63)
        ID2 = singles.tile([128, 64], f16)
        nc.vector.tensor_copy(out=ID2[0:64, :], in_=ident_h[0:64, 0:64])
        nc.vector.tensor_copy(out=ID2[64:128, :], in_=ident_h[64:128, 64:128])
        v16_t = singles.tile([128, NT], f32)
        nc.sync.dma_start(out=v16_t[:], in_=v16c[:])
        r_t = singles.tile([K, 17], f32)
        nc.sync.dma_start(out=r_t[:], in_=rois[:])
        b_t = singles.tile([K, 4, OUT_W], f32)
        nc.sync.dma_start(out=b_t[:], in_=basis[:].rearrange("k (a u) -> k a u", a=4))

        # control points: px = rois[:, 1::2]*0.25, py = rois[:, 2::2]*0.25
        px = scratch.tile([K, 8], f32, tag="px")
        py = scratch.tile([K, 8], f32, tag="py")
        r_ap = r_t[:]
        px_src = bass.AP(tensor=r_ap.tensor, offset=r_ap.offset + 1, ap=[list(r_ap.ap[0]), [2, 8]])
        py_src = bass.AP(tensor=r_ap.tensor, offset=r_ap.offset + 2, ap=[list(r_ap.ap[0]), [2, 8]])
        nc.vector.tensor_scalar(out=px[:], in0=px_src, scalar1=SCALE, scalar2=None, op0=Alu.mult)
        nc.vector.tensor_scalar(out=py[:], in0=py_src, scalar1=SCALE, scalar2=None, op0=Alu.mult)

        # curves [K, 64]: cv = sum_a B[a] * p[a(+4)]
        def bezier(dst, ptile, o):
            acc = scratch.tile([K, OUT_W], f32, tag="bzacc")
            tmp = scratch.tile([K, OUT_W], f32, tag="bztmp")
            nc.vector.tensor_scalar(out=acc[:], in0=b_t[:, 0, :], scalar1=ptile[:, o:o+1],
                                    scalar2=None, op0=Alu.mult)
            for a in (1, 2, 3):
                nc.vector.tensor_scalar(out=tmp[:], in0=b_t[:, a, :], scalar1=ptile[:, o+a:o+a+1],
                                        scalar2=None, op0=Alu.mult)
                nc.vector.tensor_tensor(out=dst[:] if a == 3 else acc[:],
                                        in0=acc[:], in1=tmp[:], op=Alu.add)

        x0 = scratch.tile([K, OUT_W], f32, tag="x0"); bezier(x0, px, 0)
        x1 = scratch.tile([K, OUT_W], f32, tag="x1"); bezier(x1, px, 4)
        y0 = scratch.tile([K, OUT_W], f32, tag="y0"); bezier(y0, py, 0)
        y1 = scratch.tile([K, OUT_W], f32, tag="y1"); bezier(y1, py, 4)

        # roi_w/h -> bwq = roi_w*0.25/64, bhq = roi_h*0.25/16  [K,1]
        def quarter_bin(ptile, scale_imm, tag):
            d1 = scratch.tile([K, 1], f32, tag=tag + "d1")
            d2 = scratch.tile([K, 1], f32, tag=tag + "d2")
            dn = scratch.tile([K, 1], f32, tag=tag + "dn")
            q = scratch.tile([K, 1], f32, tag=tag)
            nc.vector.tensor_tensor(out=d1[:], in0=ptile[:, 0:1], in1=ptile[:, 3:4], op=Alu.subtract)
            nc.vector.tensor_scalar(out=dn[:], in0=d1[:], scalar1=-1.0, scalar2=None, op0=Alu.mult)
            nc.vector.tensor_tensor(out=d1[:], in0=d1[:], in1=dn[:], op=Alu.max)
            nc.vector.tensor_tensor(out=d2[:], in0=ptile[:, 4:5], in1=ptile[:, 7:8], op=Alu.subtract)
            nc.vector.tensor_scalar(out=dn[:], in0=d2[:], scalar1=-1.0, scalar2=None, op0=Alu.mult)
            nc.vector.tensor_tensor(out=d2[:], in0=d2[:], in1=dn[:], op=Alu.max)
            nc.vector.tensor_tensor(out=d1[:], in0=d1[:], in1=d2[:], op=Alu.max)
            nc.vector.tensor_scalar(out=q[:], in0=d1[:], scalar1=scale_imm, scalar2=None, op0=Alu.mult)
            return q

        bwq = quarter_bin(px, 0.25 / OUT_W, "bwq")
        bhq = quarter_bin(py, 0.25 / OUT_H, "bhq")

        # 8 shifted curves [K, 64]: order xm0 xm1 xp0 xp1 ym0 ym1 yp0 yp1
        curves = scratch.tile([K, 9, OUT_W], f32, tag="curves")
        spec = [(x0, bwq, Alu.subtract, 0), (x1, bwq, Alu.subtract, 1),
                (x0, bwq, Alu.add, 2), (x1, bwq, Alu.add, 3),
                (y0, bhq, Alu.subtract, 4), (y1, bhq, Alu.subtract, 5),
                (y0, bhq, Alu.add, 6), (y1, bhq, Alu.add, 7)]
        for cv, qq, op, idx in spec:
            nc.vector.tensor_scalar(out=curves[:, idx, :], in0=cv[:], scalar1=qq[:, 0:1],
                                    scalar2=0.5, op0=op, op1=Alu.subtract)
        # base = batch * HW broadcast along 64
        base_c = scratch.tile([K, 1], f32, tag="base_c")
        nc.vector.tensor_scalar(out=base_c[:], in0=r_t[:, 0:1], scalar1=float(HW),
                                scalar2=None, op0=Alu.mult)
        bc_ap = base_c[:]
        nc.vector.tensor_scalar(
            out=curves[:, 8, :],
            in0=bass.AP(tensor=bc_ap.tensor, offset=bc_ap.offset, ap=[list(bc_ap.ap[0]), [0, OUT_W]]),
            scalar1=0.0, scalar2=None, op0=Alu.add)

        # transpose to TT [128, 7, K], p = iy*64 + w:
        #  slots 0-3: x endpoint curves xm0 xp0 xm1 xp1, duplicated across
        #             iy halves (x is iy-independent)
        #  slot 4: Y0 = [ym0 | yp0] iy-stacked; slot 5: Y1 = [ym1 | yp1]
        #  slot 6: base, duplicated
        TT = singles.tile([128, 7, K], f32)
        tt_spec = [(0, (0, 0)), (1, (1, 1)), (2, (2, 2)), (3, (3, 3)),
                   (4, (4, 6)), (5, (5, 7)), (6, (8, 8))]
        for slot, (qlo, qhi) in tt_spec:
            ps = pp_t.tile([128, K], f32, tag="tps", space="PSUM")
            cdup = scratch.tile([K, 128], f32, tag="cdup")
            nc.vector.tensor_copy(out=cdup[:, 0:64], in_=curves[:, qlo, :])
            nc.vector.tensor_copy(out=cdup[:, 64:128], in_=curves[:, qhi, :])
            nc.tensor.transpose(out=ps[:], in_=cdup[:], identity=ident[:K, :K])
            nc.vector.tensor_copy(out=TT[:, slot, :], in_=ps[:])

        def ttcol(q, r):
            return TT[:, q, r:r+1]

        IX, T16 = 2, NT

        def main_work():
         for r in range(K):
            # deltas [128,1]: x-lerp slopes per ix, y-lerp slope (iy-stacked)
            dxm = tabs.tile([128, 1], f32, tag="dxm")
            dxp = tabs.tile([128, 1], f32, tag="dxp")
            dyy = tabs.tile([128, 1], f32, tag="dyy")
            nc.vector.tensor_tensor(out=dxm[:], in0=ttcol(1, r), in1=ttcol(0, r), op=Alu.subtract)
            nc.vector.tensor_tensor(out=dxp[:], in0=ttcol(3, r), in1=ttcol(2, r), op=Alu.subtract)
            nc.vector.tensor_tensor(out=dyy[:], in0=ttcol(5, r), in1=ttcol(4, r), op=Alu.subtract)

            # XX [128, 2(ix), 16(t)]; YY [128, 16(t)] (iy via partition half)
            XX = tabs.tile([128, IX, T16], f32, tag="XX")
            YY = tabs.tile([128, T16], f32, tag="YY")
            nc.vector.tensor_scalar(out=XX[:, 0, :], in0=v16_t[:], scalar1=dxm[:, 0:1],
                                    scalar2=ttcol(0, r), op0=Alu.mult, op1=Alu.add)
            nc.vector.tensor_scalar(out=XX[:, 1, :], in0=v16_t[:], scalar1=dxp[:, 0:1],
                                    scalar2=ttcol(2, r), op0=Alu.mult, op1=Alu.add)
            nc.vector.tensor_scalar(out=YY[:], in0=v16_t[:], scalar1=dyy[:, 0:1],
                                    scalar2=ttcol(4, r), op0=Alu.mult, op1=Alu.add)

            # coord pipe
            def pipe(P, F, limit, tagp):
                vv = tabs.tile([128, F], f32, tag=tagp + "v")
                v2 = tabs.tile([128, F], f32, tag=tagp + "v2")
                xx = tabs.tile([128, F], f32, tag=tagp + "x")
                xi = tabs.tile([128, F], i32, tag=tagp + "i")
                xf = tabs.tile([128, F], f32, tag=tagp + "f")
                xfc = tabs.tile([128, F], f32, tag=tagp + "fc")
                lo = tabs.tile([128, F], f32, tag=tagp + "lo")
                mm = tabs.tile([128, F], f32, tag=tagp + "m")
                lx = tabs.tile([128, F], f32, tag=tagp + "l")
                hx = tabs.tile([128, F], f32, tag=tagp + "h")
                nc.vector.tensor_scalar(out=vv[:], in0=P, scalar1=-1.0, scalar2=None, op0=Alu.is_gt)
                nc.vector.tensor_scalar(out=v2[:], in0=P, scalar1=float(limit), scalar2=None, op0=Alu.is_lt)
                nc.vector.tensor_tensor(out=vv[:], in0=vv[:], in1=v2[:], op=Alu.mult)
                nc.vector.tensor_scalar(out=xx[:], in0=P, scalar1=0.0, scalar2=None, op0=Alu.max)
                nc.vector.tensor_scalar(out=xi[:], in0=xx[:], scalar1=0.5, scalar2=None, op0=Alu.subtract)
                nc.vector.tensor_copy(out=xf[:], in_=xi[:])
                nc.vector.tensor_scalar(out=xfc[:], in0=xf[:], scalar1=float(limit - 1),
                                        scalar2=None, op0=Alu.min)
                nc.vector.tensor_tensor(out=lo[:], in0=xx[:], in1=xfc[:], op=Alu.subtract)
                nc.vector.tensor_scalar(out=mm[:], in0=xfc[:], scalar1=float(limit - 1),
                                        scalar2=None, op0=Alu.is_lt)
                nc.vector.tensor_tensor(out=lx[:], in0=lo[:], in1=mm[:], op=Alu.mult)
                nc.vector.tensor_scalar(out=hx[:], in0=lx[:], scalar1=-1.0, scalar2=1.0,
                                        op0=Alu.mult, op1=Alu.add)
                return vv, xfc, lx, hx

            vx, xfc, lx, hx = pipe(XX[:].rearrange("p a t -> p (a t)"), IX * T16, W, "px")
            vy, yfc, ly, hy = pipe(YY[:], T16, H, "py")

            # per-sample x validity fold: hxv = hx*vx, lxv = lx*vx [128, 32]
            hxv = tabs.tile([128, IX * T16], f32, tag="hxv")
            lxv = tabs.tile([128, IX * T16], f32, tag="lxv")
            nc.vector.tensor_tensor(out=hxv[:], in0=hx[:], in1=vx[:], op=Alu.mult)
            nc.vector.tensor_tensor(out=lxv[:], in0=lx[:], in1=vx[:], op=Alu.mult)

            # window-position indicators: d1 = xfc1 - xfc0 in {0,1,2} [128,16]
            d1 = tabs.tile([128, T16], f32, tag="d1")
            nc.vector.tensor_tensor(out=d1[:], in0=xfc[:, T16:2*T16], in1=xfc[:, 0:T16],
                                    op=Alu.subtract)
            i0 = tabs.tile([128, T16], f32, tag="i0")
            i1 = tabs.tile([128, T16], f32, tag="i1")
            i2 = tabs.tile([128, T16], f32, tag="i2")
            nc.vector.tensor_scalar(out=i0[:], in0=d1[:], scalar1=0.5, scalar2=None, op0=Alu.is_lt)
            nc.vector.tensor_scalar(out=i2[:], in0=d1[:], scalar1=1.5, scalar2=None, op0=Alu.is_gt)
            nc.vector.tensor_tensor(out=i1[:], in0=i0[:], in1=i2[:], op=Alu.add)
            nc.vector.tensor_scalar(out=i1[:], in0=i1[:], scalar1=-1.0, scalar2=1.0,
                                    op0=Alu.mult, op1=Alu.add)

            # x-weight planes WXT [128, 4(dx), 16(t)]
            # wx0 = hxv0 + i0*hxv1 ; wx1 = lxv0 + i0*lxv1 + i1*hxv1
            # wx2 = i1*lxv1 + i2*hxv1 ; wx3 = i2*lxv1
            WXT = tabs.tile([128, 4, T16], f32, tag="WXT")
            tmpa = tabs.tile([128, T16], f32, tag="tmpa")
            tmpb = tabs.tile([128, T16], f32, tag="tmpb")
            h1 = hxv[:, T16:2*T16]
            l1 = lxv[:, T16:2*T16]
            nc.vector.tensor_tensor(out=tmpa[:], in0=i0[:], in1=h1, op=Alu.mult)
            nc.vector.tensor_tensor(out=WXT[:, 0, :], in0=hxv[:, 0:T16], in1=tmpa[:], op=Alu.add)
            nc.vector.tensor_tensor(out=tmpa[:], in0=i0[:], in1=l1, op=Alu.mult)
            nc.vector.tensor_tensor(out=tmpb[:], in0=i1[:], in1=h1, op=Alu.mult)
            nc.vector.tensor_tensor(out=tmpa[:], in0=tmpa[:], in1=tmpb[:], op=Alu.add)
            nc.vector.tensor_tensor(out=WXT[:, 1, :], in0=lxv[:, 0:T16], in1=tmpa[:], op=Alu.add)
            nc.vector.tensor_tensor(out=tmpa[:], in0=i1[:], in1=l1, op=Alu.mult)
            nc.vector.tensor_tensor(out=tmpb[:], in0=i2[:], in1=h1, op=Alu.mult)
            nc.vector.tensor_tensor(out=WXT[:, 2, :], in0=tmpa[:], in1=tmpb[:], op=Alu.add)
            nc.vector.tensor_tensor(out=WXT[:, 3, :], in0=i2[:], in1=l1, op=Alu.mult)

            # offsets: o = (yfc*W + base) + xfc0  [128, 16] -> int32
            yw = tabs.tile([128, T16], f32, tag="yw")
            nc.vector.tensor_scalar(out=yw[:], in0=yfc[:], scalar1=float(W),
                                    scalar2=ttcol(6, r), op0=Alu.mult, op1=Alu.add)
            of = tabs.tile([128, T16], f32, tag="of")
            nc.vector.tensor_tensor(out=of[:], in0=yw[:], in1=xfc[:, 0:T16], op=Alu.add)
            O = tabs.tile([128, T16], i32, tag="O")
            nc.vector.tensor_copy(out=O[:], in_=of[:])

            # y weights folded with valid & 1/4: hyq = hy*vy*0.25, lyq likewise
            q0 = tabs.tile([128, T16], f32, tag="q0")
            nc.vector.tensor_scalar(out=q0[:], in0=vy[:], scalar1=0.25, scalar2=None, op0=Alu.mult)
            hyq = tabs.tile([128, T16], f32, tag="hyq")
            lyq = tabs.tile([128, T16], f32, tag="lyq")
            nc.vector.tensor_tensor(out=hyq[:], in0=hy[:], in1=q0[:], op=Alu.mult)
            nc.vector.tensor_tensor(out=lyq[:], in0=ly[:], in1=q0[:], op=Alu.mult)

            # W8 [128, 8 (2dx+ylh), 16 (t)]: wx[dx] x (hyq, lyq)
            W8 = tabs.tile([128, NB, T16], f32, tag="W8")
            for dx in range(4):
                nc.vector.tensor_tensor(out=W8[:, 2*dx, :], in0=hyq[:], in1=WXT[:, dx, :], op=Alu.mult)
                nc.vector.tensor_tensor(out=W8[:, 2*dx+1, :], in0=lyq[:], in1=WXT[:, dx, :], op=Alu.mult)

            # per tile (one output row, 64 bins): gather + combine into
            # [C-half, 64] PSUM via double-diag matmuls
            st = None
            for t in range(T16):
                g = gpool.tile([128, NB * C], f16, tag="g")
                nc.gpsimd.indirect_dma_start(
                    out=g[:], out_offset=None, in_=feat8[:],
                    in_offset=bass.IndirectOffsetOnAxis(ap=O[:, t:t+1], axis=0))
                # double-diags for this tile, fused in PAIRS (128-el DVE ops;
                # wider fusions slow the indirect gathers via SBUF contention):
                # DG8[p, blk, b] = ID2[p, b] * W8[p, blk, t]
                DG8 = dpool.tile([128, NB, 64], f16, tag="DG8")
                id_ap = ID2[:]
                id_b = bass.AP(tensor=id_ap.tensor, offset=id_ap.offset,
                               ap=[list(id_ap.ap[0]), [0, 2], [1, 64]])
                w8_ap = W8[:]
                for bp in range(NB // 2):
                    w8_b = bass.AP(tensor=w8_ap.tensor,
                                   offset=w8_ap.offset + (2 * bp) * T16 + t,
                                   ap=[list(w8_ap.ap[0]), [T16, 2], [0, 64]])
                    nc.vector.tensor_tensor(out=DG8[:, 2*bp:2*bp+2, :],
                                            in0=id_b, in1=w8_b, op=Alu.mult)
                psA = pp_mm.tile([128, 64], f32, tag="psA", space="PSUM")
                psB = pp_mm.tile([128, 64], f32, tag="psB", space="PSUM")
                for blk in range(NB):
                    nc.tensor.matmul(psA[:], lhsT=g[:, blk*C:blk*C+128],
                                     rhs=DG8[:, blk, :],
                                     start=(blk == 0), stop=(blk == NB - 1))
                    nc.tensor.matmul(psB[:], lhsT=g[:, blk*C+128:blk*C+256],
                                     rhs=DG8[:, blk, :],
                                     start=(blk == 0), stop=(blk == NB - 1))
                # stage 2 tiles per out-DMA (512B runs per partition)
                if t % 2 == 0:
                    st = spool.tile([128, 2, 2, 64], f32, tag="st")
                nc.scalar.copy(st[:, 0, t % 2, :], psA[:])
                nc.scalar.copy(st[:, 1, t % 2, :], psB[:])
                if t % 2 == 1:
                    nc.sync.dma_start(out=out_v[r, :, :, t-1:t+1, :], in_=st[:])

        if nrep > 1:
            with tc.For_i(0, nrep, 1):
                main_work()
        else:
            main_work()

    nc.finalize()
    return nc


def _get_nc():
    if "nc" not in _CACHE:
        _CACHE["nc"] = _build_nc()
    return _CACHE["nc"]


def run_sharded(input, rois, **spmd_kwargs):
    """Run on 8 cores; returns (full_output, BassKernelResults)."""
    from concourse.bass_utils import run_bass_kernel_spmd

    x = np.ascontiguousarray(np.asarray(input, dtype=np.float32))
    rr = np.ascontiguousarray(np.asarray(rois, dtype=np.float32))
    feat8 = _build_feat8(x)
    basis32, v16 = _host_constants()

    in_maps = []
    for c in range(NCORES):
        in_maps.append({
            "feat8": feat8,
            "rois": np.ascontiguousarray(rr[c*K:(c+1)*K]),
            "basis": basis32,
            "v16c": v16,
        })
    nc = _get_nc()
    res = run_bass_kernel_spmd(nc, in_maps, core_ids=list(range(NCORES)), **spmd_kwargs)
    outp = np.concatenate([res.results[c]["out"] for c in range(NCORES)], axis=0)
    return outp, res


def kernel(input, rois):
    out, _ = run_sharded(input, rois)
    return out


# revision 23
# speedup vs baseline: 1.1148x; 1.0194x over previous
"""BezierAlign Trainium2 kernel.

Full inputs -> full output. Shards the R=256 ROIs across 8 NeuronCores (32
ROIs/core); the feature map is replicated to every core in a "window block"
layout: block(n,y,x) holds the 4-wide x 2-tall fp16 pixel window
[f(y,x..x+3) x f(y..y+1)] = 8C values = 4KB, so ONE indirect-DMA descriptor
fetches the footprint of BOTH x-samples of a bin row (max bin width 2.5 px
=> x_low spread <= 2). Indirect gathers are descriptor-rate-bound (~8.6 ns
per descriptor), so descriptor count is what matters.

Partition layout packs the TWO y-sample rows into the matmul contraction:
p = iy*64 + w (iy = y-sample row, w = output column). A tile is one output
row (64 bins). One gather per tile fetches 128 rows = both y-samples of all
64 bins; one matmul per (window-block, C-half) contracts both y-samples
via a stacked double-diagonal rhs [128, 64], so the PE streams only 64
columns per matmul (16 col/bin total vs 32 for the naive scheme) and the
diag-build DVE work is halved.

Per-core device program:
  1. Evaluate the 4 cubic Bezier curves per ROI on 32 partitions (roi-major),
     fold the +-0.25*bin sample offsets and the -0.5 align shift into shifted
     endpoint curves, PE-transpose them (x-curves duplicated across halves,
     y-curves iy-stacked) to the p = (iy, w) layout.
  2. Per ROI, compute sample coords / validity / per-window weights W8
     (4 x-positions x 2 y-rows, with dx-selection masks merging the two
     x-samples) / gather offsets, all on (iy, w) partitions, f32 DVE.
  3. Per 64-bin tile: 1 indirect gather ([128, 8C] fp16), 8 double-diag
     builds, 16 fp16 matmuls accumulating [C-half, 64 bins] in 2 PSUM
     tiles; copy to SBUF, DMA out.
"""

import numpy as np

# problem shapes (hardcoded per contract)
N, C, H, W = 2, 256, 160, 160
R = 256
OUT_H, OUT_W = 16, 64
SCALE = 0.25
NCORES = 8
K = R // NCORES          # 32 rois per core
NT = OUT_H               # 16 tiles of 64 bins (one output row) per roi
HW = H * W
NB = 8                   # blocks per window: 4 dx * 2 y

_CACHE = {}


def _host_constants():
    f32 = np.float32
    u = (np.arange(OUT_W, dtype=f32) / f32(OUT_W)).astype(f32)
    mt = (f32(1.0) - u).astype(f32)
    basis = np.stack([mt**3, 3 * u * mt**2, 3 * u**2 * mt, u**3]).astype(f32)  # [4,64]
    basis32 = np.broadcast_to(basis.reshape(1, 4 * OUT_W), (K, 4 * OUT_W)).copy()
    t = np.arange(NT, dtype=f32) / f32(NT)
    v16 = np.broadcast_to(t[None, :], (128, NT)).copy()      # [128,16] row v
    return basis32, v16


def _build_feat8(x):
    """x [N, C, H, W] f32 -> fp16 [N*H*W, 8C]; block(n,y,x) =
    [f(y,x), f(y+1,x), f(y,x+1), f(y+1,x+1), ..., f(y,x+3), f(y+1,x+3)]
    with out-of-image parts zeroed."""
    f = np.ascontiguousarray(x.transpose(0, 2, 3, 1)).astype(np.float16)  # [N,H,W,C]
    fy = np.zeros_like(f)
    fy[:, :-1] = f[:, 1:]
    a = np.concatenate([f, fy], axis=-1)                  # [N,H,W,2C] y-pair
    parts = [a]
    for dx in (1, 2, 3):
        ax = np.zeros_like(a)
        ax[:, :, :-dx] = a[:, :, dx:]
        parts.append(ax)
    feat8 = np.concatenate(parts, axis=-1)                # [N,H,W,8C]
    return np.ascontiguousarray(feat8.reshape(N * HW, NB * C))


def _build_nc(nrep=1):
    from contextlib import ExitStack
    import concourse.bacc as bacc
    import concourse.bass as bass
    import concourse.tile as tile
    from concourse import mybir
    from concourse.masks import make_identity

    f32 = mybir.dt.float32
    f16 = mybir.dt.float16
    i32 = mybir.dt.int32
    Alu = mybir.AluOpType

    nc = bacc.Bacc(None, target_bir_lowering=False)

    feat8 = nc.dram_tensor("feat8", [N * HW, NB * C], f16, kind="ExternalInput")
    rois = nc.dram_tensor("rois", [K, 17], f32, kind="ExternalInput")
    basis = nc.dram_tensor("basis", [K, 4 * OUT_W], f32, kind="ExternalInput")
    v16c = nc.dram_tensor("v16c", [128, NT], f32, kind="ExternalInput")
    out = nc.dram_tensor("out", [K, C, OUT_H, OUT_W], f32, kind="ExternalOutput")
    # [K, C, 16, 64] -> (k, p, h, t, w): c = h*128 + p
    out_v = out.rearrange("k (h p) t w -> k p h t w", h=2)

    with tile.TileContext(nc) as tc, ExitStack() as ctx:
        singles = ctx.enter_context(tc.tile_pool(name="singles", bufs=1))
        scratch = ctx.enter_context(tc.tile_pool(name="scratch", bufs=2))
        tabs = ctx.enter_context(tc.tile_pool(name="tabs", bufs=4))
        gpool = ctx.enter_context(tc.tile_pool(name="gpool", bufs=8))
        dpool = ctx.enter_context(tc.tile_pool(name="dpool", bufs=24))
        spool = ctx.enter_context(tc.tile_pool(name="spool", bufs=6))
        pp_t = ctx.enter_context(tc.tile_pool(name="pp_t", bufs=1, space="PSUM"))
        pp_mm = ctx.enter_context(tc.tile_pool(name="pp_mm", bufs=3, space="PSUM"))

        ident = singles.tile([128, 128], f32)
        make_identity(nc, ident[:])
        ident_h = singles.tile([128, 128], f16)
        nc.vector.tensor_copy(out=ident_h[:], in_=ident[:])
        # ID2 [128, 64] fp16: stacked pair of 64-identities (rows 0-63 and
        # 64-127 both diag on columns 0-63)
        ID2 = singles.tile([128, 64], f16)
        nc.vector.tensor_copy(out=ID2[0:64, :], in_=ident_h[0:64, 0:64])
        nc.vector.tensor_copy(out=ID2[64:128, :], in_=ident_h[64:128, 64:128])
        v16_t = singles.tile([128, NT], f32)
        nc.sync.dma_start(out=v16_t[:], in_=v16c[:])
        r_t = singles.tile([K, 17], f32)
        nc.sync.dma_start(out=r_t[:], in_=rois[:])
        b_t = singles.tile([K, 4, OUT_W], f32)
        nc.sync.dma_start(out=b_t[:], in_=basis[:].rearrange("k (a u) -> k a u", a=4))

        # control points: px = rois[:, 1::2]*0.25, py = rois[:, 2::2]*0.25
        px = scratch.tile([K, 8], f32, tag="px")
        py = scratch.tile([K, 8], f32, tag="py")
        r_ap = r_t[:]
        px_src = bass.AP(tensor=r_ap.tensor, offset=r_ap.offset + 1, ap=[list(r_ap.ap[0]), [2, 8]])
        py_src = bass.AP(tensor=r_ap.tensor, offset=r_ap.offset + 2, ap=[list(r_ap.ap[0]), [2, 8]])
        nc.vector.tensor_scalar(out=px[:], in0=px_src, scalar1=SCALE, scalar2=None, op0=Alu.mult)
        nc.vector.tensor_scalar(out=py[:], in0=py_src, scalar1=SCALE, scalar2=None, op0=Alu.mult)

        # curves [K, 64]: cv = sum_a B[a] * p[a(+4)]
        def bezier(dst, ptile, o):
            acc = scratch.tile([K, OUT_W], f32, tag="bzacc")
            tmp = scratch.tile([K, OUT_W], f32, tag="bztmp")
            nc.vector.tensor_scalar(out=acc[:], in0=b_t[:, 0, :], scalar1=ptile[:, o:o+1],
                                    scalar2=None, op0=Alu.mult)
            for a in (1, 2, 3):
                nc.vector.tensor_scalar(out=tmp[:], in0=b_t[:, a, :], scalar1=ptile[:, o+a:o+a+1],
                                        scalar2=None, op0=Alu.mult)
                nc.vector.tensor_tensor(out=dst[:] if a == 3 else acc[:],
                                        in0=acc[:], in1=tmp[:], op=Alu.add)

        x0 = scratch.tile([K, OUT_W], f32, tag="x0"); bezier(x0, px, 0)
        x1 = scratch.tile([K, OUT_W], f32, tag="x1"); bezier(x1, px, 4)
        y0 = scratch.tile([K, OUT_W], f32, tag="y0"); bezier(y0, py, 0)
        y1 = scratch.tile([K, OUT_W], f32, tag="y1"); bezier(y1, py, 4)

        # roi_w/h -> bwq = roi_w*0.25/64, bhq = roi_h*0.25/16  [K,1]
        def quarter_bin(ptile, scale_imm, tag):
            d1 = scratch.tile([K, 1], f32, tag=tag + "d1")
            d2 = scratch.tile([K, 1], f32, tag=tag + "d2")
            dn = scratch.tile([K, 1], f32, tag=tag + "dn")
            q = scratch.tile([K, 1], f32, tag=tag)
            nc.vector.tensor_tensor(out=d1[:], in0=ptile[:, 0:1], in1=ptile[:, 3:4], op=Alu.subtract)
            nc.vector.tensor_scalar(out=dn[:], in0=d1[:], scalar1=-1.0, scalar2=None, op0=Alu.mult)
            nc.vector.tensor_tensor(out=d1[:], in0=d1[:], in1=dn[:], op=Alu.max)
            nc.vector.tensor_tensor(out=d2[:], in0=ptile[:, 4:5], in1=ptile[:, 7:8], op=Alu.subtract)
            nc.vector.tensor_scalar(out=dn[:], in0=d2[:], scalar1=-1.0, scalar2=None, op0=Alu.mult)
            nc.vector.tensor_tensor(out=d2[:], in0=d2[:], in1=dn[:], op=Alu.max)
            nc.vector.tensor_tensor(out=d1[:], in0=d1[:], in1=d2[:], op=Alu.max)
            nc.vector.tensor_scalar(out=q[:], in0=d1[:], scalar1=scale_imm, scalar2=None, op0=Alu.mult)
            return q

        bwq = quarter_bin(px, 0.25 / OUT_W, "bwq")
        bhq = quarter_bin(py, 0.25 / OUT_H, "bhq")

        # 8 shifted curves [K, 64]: order xm0 xm1 xp0 xp1 ym0 ym1 yp0 yp1
        curves = scratch.tile([K, 9, OUT_W], f32, tag="curves")
        spec = [(x0, bwq, Alu.subtract, 0), (x1, bwq, Alu.subtract, 1),
                (x0, bwq, Alu.add, 2), (x1, bwq, Alu.add, 3),
                (y0, bhq, Alu.subtract, 4), (y1, bhq, Alu.subtract, 5),
                (y0, bhq, Alu.add, 6), (y1, bhq, Alu.add, 7)]
        for cv, qq, op, idx in spec:
            nc.vector.tensor_scalar(out=curves[:, idx, :], in0=cv[:], scalar1=qq[:, 0:1],
                                    scalar2=0.5, op0=op, op1=Alu.subtract)
        # base = batch * HW broadcast along 64
        base_c = scratch.tile([K, 1], f32, tag="base_c")
        nc.vector.tensor_scalar(out=base_c[:], in0=r_t[:, 0:1], scalar1=float(HW),
                                scalar2=None, op0=Alu.mult)
        bc_ap = base_c[:]
        nc.vector.tensor_scalar(
            out=curves[:, 8, :],
            in0=bass.AP(tensor=bc_ap.tensor, offset=bc_ap.offset, ap=[list(bc_ap.ap[0]), [0, OUT_W]]),
            scalar1=0.0, scalar2=None, op0=Alu.add)

        # transpose to TT [128, 7, K], p = iy*64 + w:
        #  slots 0-3: x endpoint curves xm0 xp0 xm1 xp1, duplicated across
        #             iy halves (x is iy-independent)
        #  slot 4: Y0 = [ym0 | yp0] iy-stacked; slot 5: Y1 = [ym1 | yp1]
        #  slot 6: base, duplicated
        TT = singles.tile([128, 7, K], f32)
        tt_spec = [(0, (0, 0)), (1, (1, 1)), (2, (2, 2)), (3, (3, 3)),
                   (4, (4, 6)), (5, (5, 7)), (6, (8, 8))]
        for slot, (qlo, qhi) in tt_spec:
            ps = pp_t.tile([128, K], f32, tag="tps", space="PSUM")
            cdup = scratch.tile([K, 128], f32, tag="cdup")
            nc.vector.tensor_copy(out=cdup[:, 0:64], in_=curves[:, qlo, :])
            nc.vector.tensor_copy(out=cdup[:, 64:128], in_=curves[:, qhi, :])
            nc.tensor.transpose(out=ps[:], in_=cdup[:], identity=ident[:K, :K])
            nc.vector.tensor_copy(out=TT[:, slot, :], in_=ps[:])

        def ttcol(q, r):
            return TT[:, q, r:r+1]

        IX, T16 = 2, NT

        def do_pipe(r):
            # deltas [128,1]: x-lerp slopes per ix, y-lerp slope (iy-stacked)
            dxm = tabs.tile([128, 1], f32, tag="dxm")
            dxp = tabs.tile([128, 1], f32, tag="dxp")
            dyy = tabs.tile([128, 1], f32, tag="dyy")
            nc.vector.tensor_tensor(out=dxm[:], in0=ttcol(1, r), in1=ttcol(0, r), op=Alu.subtract)
            nc.vector.tensor_tensor(out=dxp[:], in0=ttcol(3, r), in1=ttcol(2, r), op=Alu.subtract)
            nc.vector.tensor_tensor(out=dyy[:], in0=ttcol(5, r), in1=ttcol(4, r), op=Alu.subtract)

            # XX [128, 2(ix), 16(t)]; YY [128, 16(t)] (iy via partition half)
            XX = tabs.tile([128, IX, T16], f32, tag="XX")
            YY = tabs.tile([128, T16], f32, tag="YY")
            nc.vector.tensor_scalar(out=XX[:, 0, :], in0=v16_t[:], scalar1=dxm[:, 0:1],
                                    scalar2=ttcol(0, r), op0=Alu.mult, op1=Alu.add)
            nc.vector.tensor_scalar(out=XX[:, 1, :], in0=v16_t[:], scalar1=dxp[:, 0:1],
                                    scalar2=ttcol(2, r), op0=Alu.mult, op1=Alu.add)
            nc.vector.tensor_scalar(out=YY[:], in0=v16_t[:], scalar1=dyy[:, 0:1],
                                    scalar2=ttcol(4, r), op0=Alu.mult, op1=Alu.add)

            # coord pipe
            def pipe(P, F, limit, tagp):
                vv = tabs.tile([128, F], f32, tag=tagp + "v")
                v2 = tabs.tile([128, F], f32, tag=tagp + "v2")
                xx = tabs.tile([128, F], f32, tag=tagp + "x")
                xi = tabs.tile([128, F], i32, tag=tagp + "i")
                xf = tabs.tile([128, F], f32, tag=tagp + "f")
                xfc = tabs.tile([128, F], f32, tag=tagp + "fc")
                lo = tabs.tile([128, F], f32, tag=tagp + "lo")
                mm = tabs.tile([128, F], f32, tag=tagp + "m")
                lx = tabs.tile([128, F], f32, tag=tagp + "l")
                hx = tabs.tile([128, F], f32, tag=tagp + "h")
                nc.vector.tensor_scalar(out=vv[:], in0=P, scalar1=-1.0, scalar2=None, op0=Alu.is_gt)
                nc.vector.tensor_scalar(out=v2[:], in0=P, scalar1=float(limit), scalar2=None, op0=Alu.is_lt)
                nc.vector.tensor_tensor(out=vv[:], in0=vv[:], in1=v2[:], op=Alu.mult)
                nc.vector.tensor_scalar(out=xx[:], in0=P, scalar1=0.0, scalar2=None, op0=Alu.max)
                nc.vector.tensor_scalar(out=xi[:], in0=xx[:], scalar1=0.5, scalar2=None, op0=Alu.subtract)
                nc.vector.tensor_copy(out=xf[:], in_=xi[:])
                nc.vector.tensor_scalar(out=xfc[:], in0=xf[:], scalar1=float(limit - 1),
                                        scalar2=None, op0=Alu.min)
                nc.vector.tensor_tensor(out=lo[:], in0=xx[:], in1=xfc[:], op=Alu.subtract)
                nc.vector.tensor_scalar(out=mm[:], in0=xfc[:], scalar1=float(limit - 1),
                                        scalar2=None, op0=Alu.is_lt)
                nc.vector.tensor_tensor(out=lx[:], in0=lo[:], in1=mm[:], op=Alu.mult)
                nc.vector.tensor_scalar(out=hx[:], in0=lx[:], scalar1=-1.0, scalar2=1.0,
                                        op0=Alu.mult, op1=Alu.add)
                return vv, xfc, lx, hx

            vx, xfc, lx, hx = pipe(XX[:].rearrange("p a t -> p (a t)"), IX * T16, W, "px")
            vy, yfc, ly, hy = pipe(YY[:], T16, H, "py")

            # per-sample x validity fold: hxv = hx*vx, lxv = lx*vx [128, 32]
            hxv = tabs.tile([128, IX * T16], f32, tag="hxv")
            lxv = tabs.tile([128, IX * T16], f32, tag="lxv")
            nc.vector.tensor_tensor(out=hxv[:], in0=hx[:], in1=vx[:], op=Alu.mult)
            nc.vector.tensor_tensor(out=lxv[:], in0=lx[:], in1=vx[:], op=Alu.mult)

            # window-position indicators: d1 = xfc1 - xfc0 in {0,1,2} [128,16]
            d1 = tabs.tile([128, T16], f32, tag="d1")
            nc.vector.tensor_tensor(out=d1[:], in0=xfc[:, T16:2*T16], in1=xfc[:, 0:T16],
                                    op=Alu.subtract)
            i0 = tabs.tile([128, T16], f32, tag="i0")
            i1 = tabs.tile([128, T16], f32, tag="i1")
            i2 = tabs.tile([128, T16], f32, tag="i2")
            nc.vector.tensor_scalar(out=i0[:], in0=d1[:], scalar1=0.5, scalar2=None, op0=Alu.is_lt)
            nc.vector.tensor_scalar(out=i2[:], in0=d1[:], scalar1=1.5, scalar2=None, op0=Alu.is_gt)
            nc.vector.tensor_tensor(out=i1[:], in0=i0[:], in1=i2[:], op=Alu.add)
            nc.vector.tensor_scalar(out=i1[:], in0=i1[:], scalar1=-1.0, scalar2=1.0,
                                    op0=Alu.mult, op1=Alu.add)

            # x-weight planes WXT [128, 4(dx), 16(t)]
            # wx0 = hxv0 + i0*hxv1 ; wx1 = lxv0 + i0*lxv1 + i1*hxv1
            # wx2 = i1*lxv1 + i2*hxv1 ; wx3 = i2*lxv1
            WXT = tabs.tile([128, 4, T16], f32, tag="WXT")
            tmpa = tabs.tile([128, T16], f32, tag="tmpa")
            tmpb = tabs.tile([128, T16], f32, tag="tmpb")
            h1 = hxv[:, T16:2*T16]
            l1 = lxv[:, T16:2*T16]
            nc.vector.tensor_tensor(out=tmpa[:], in0=i0[:], in1=h1, op=Alu.mult)
            nc.vector.tensor_tensor(out=WXT[:, 0, :], in0=hxv[:, 0:T16], in1=tmpa[:], op=Alu.add)
            nc.vector.tensor_tensor(out=tmpa[:], in0=i0[:], in1=l1, op=Alu.mult)
            nc.vector.tensor_tensor(out=tmpb[:], in0=i1[:], in1=h1, op=Alu.mult)
            nc.vector.tensor_tensor(out=tmpa[:], in0=tmpa[:], in1=tmpb[:], op=Alu.add)
            nc.vector.tensor_tensor(out=WXT[:, 1, :], in0=lxv[:, 0:T16], in1=tmpa[:], op=Alu.add)
            nc.vector.tensor_tensor(out=tmpa[:], in0=i1[:], in1=l1, op=Alu.mult)
            nc.vector.tensor_tensor(out=tmpb[:], in0=i2[:], in1=h1, op=Alu.mult)
            nc.vector.tensor_tensor(out=WXT[:, 2, :], in0=tmpa[:], in1=tmpb[:], op=Alu.add)
            nc.vector.tensor_tensor(out=WXT[:, 3, :], in0=i2[:], in1=l1, op=Alu.mult)

            # offsets: o = (yfc*W + base) + xfc0  [128, 16] -> int32
            yw = tabs.tile([128, T16], f32, tag="yw")
            nc.vector.tensor_scalar(out=yw[:], in0=yfc[:], scalar1=float(W),
                                    scalar2=ttcol(6, r), op0=Alu.mult, op1=Alu.add)
            of = tabs.tile([128, T16], f32, tag="of")
            nc.vector.tensor_tensor(out=of[:], in0=yw[:], in1=xfc[:, 0:T16], op=Alu.add)
            O = tabs.tile([128, T16], i32, tag="O")
            nc.vector.tensor_copy(out=O[:], in_=of[:])

            # y weights folded with valid & 1/4: hyq = hy*vy*0.25, lyq likewise
            q0 = tabs.tile([128, T16], f32, tag="q0")
            nc.vector.tensor_scalar(out=q0[:], in0=vy[:], scalar1=0.25, scalar2=None, op0=Alu.mult)
            hyq = tabs.tile([128, T16], f32, tag="hyq")
            lyq = tabs.tile([128, T16], f32, tag="lyq")
            nc.vector.tensor_tensor(out=hyq[:], in0=hy[:], in1=q0[:], op=Alu.mult)
            nc.vector.tensor_tensor(out=lyq[:], in0=ly[:], in1=q0[:], op=Alu.mult)

            # W8 [128, 8 (2dx+ylh), 16 (t)]: wx[dx] x (hyq, lyq)
            W8 = tabs.tile([128, NB, T16], f32, tag="W8")
            for dx in range(4):
                nc.vector.tensor_tensor(out=W8[:, 2*dx, :], in0=hyq[:], in1=WXT[:, dx, :], op=Alu.mult)
                nc.vector.tensor_tensor(out=W8[:, 2*dx+1, :], in0=lyq[:], in1=WXT[:, dx, :], op=Alu.mult)
            return O, W8

        def do_tiles(r, O, W8):
            # per tile (one output row, 64 bins): gather + combine into
            # [C-half, 64] PSUM via double-diag matmuls
            st = None
            for t in range(T16):
                g = gpool.tile([128, NB * C], f16, tag="g")
                nc.gpsimd.indirect_dma_start(
                    out=g[:], out_offset=None, in_=feat8[:],
                    in_offset=bass.IndirectOffsetOnAxis(ap=O[:, t:t+1], axis=0))
                # double-diags for this tile, fused in PAIRS (128-el DVE ops;
                # wider fusions slow the indirect gathers via SBUF contention):
                # DG8[p, blk, b] = ID2[p, b] * W8[p, blk, t]
                DG8 = dpool.tile([128, NB, 64], f16, tag="DG8")
                id_ap = ID2[:]
                id_b = bass.AP(tensor=id_ap.tensor, offset=id_ap.offset,
                               ap=[list(id_ap.ap[0]), [0, 2], [1, 64]])
                w8_ap = W8[:]
                for bp in range(NB // 2):
                    w8_b = bass.AP(tensor=w8_ap.tensor,
                                   offset=w8_ap.offset + (2 * bp) * T16 + t,
                                   ap=[list(w8_ap.ap[0]), [T16, 2], [0, 64]])
                    nc.vector.tensor_tensor(out=DG8[:, 2*bp:2*bp+2, :],
                                            in0=id_b, in1=w8_b, op=Alu.mult)
                psA = pp_mm.tile([128, 64], f32, tag="psA", space="PSUM")
                psB = pp_mm.tile([128, 64], f32, tag="psB", space="PSUM")
                for blk in range(NB):
                    nc.tensor.matmul(psA[:], lhsT=g[:, blk*C:blk*C+128],
                                     rhs=DG8[:, blk, :],
                                     start=(blk == 0), stop=(blk == NB - 1))
                    nc.tensor.matmul(psB[:], lhsT=g[:, blk*C+128:blk*C+256],
                                     rhs=DG8[:, blk, :],
                                     start=(blk == 0), stop=(blk == NB - 1))
                # stage 2 tiles per out-DMA (512B runs per partition)
                if t % 2 == 0:
                    st = spool.tile([128, 2, 2, 64], f32, tag="st")
                nc.scalar.copy(st[:, 0, t % 2, :], psA[:])
                nc.scalar.copy(st[:, 1, t % 2, :], psB[:])
                if t % 2 == 1:
                    nc.sync.dma_start(out=out_v[r, :, :, t-1:t+1, :], in_=st[:])

        def main_work():
            # software-pipeline: emit roi r+1's coordinate pipe BEFORE roi
            # r's tile crunch so O/W8 are ready when the gathers drain
            pend = do_pipe(0)
            for r in range(K):
                cur = pend
                if r + 1 < K:
                    pend = do_pipe(r + 1)
                do_tiles(r, *cur)

        if nrep > 1:
            with tc.For_i(0, nrep, 1):
                main_work()
        else:
            main_work()

    nc.finalize()
    return nc


def _get_nc():
    if "nc" not in _CACHE:
        _CACHE["nc"] = _build_nc()
    return _CACHE["nc"]


def run_sharded(input, rois, **spmd_kwargs):
    """Run on 8 cores; returns (full_output, BassKernelResults)."""
    from concourse.bass_utils import run_bass_kernel_spmd

    x = np.ascontiguousarray(np.asarray(input, dtype=np.float32))
    rr = np.ascontiguousarray(np.asarray(rois, dtype=np.float32))
    feat8 = _build_feat8(x)
    basis32, v16 = _host_constants()

    in_maps = []
    for c in range(NCORES):
        in_maps.append({
            "feat8": feat8,
            "rois": np.ascontiguousarray(rr[c*K:(c+1)*K]),
            "basis": basis32,
            "v16c": v16,
        })
    nc = _get_nc()
    res = run_bass_kernel_spmd(nc, in_maps, core_ids=list(range(NCORES)), **spmd_kwargs)
    outp = np.concatenate([res.results[c]["out"] for c in range(NCORES)], axis=0)
    return outp, res


def kernel(input, rois):
    out, _ = run_sharded(input, rois)
    return out


# revision 30
# speedup vs baseline: 1.1409x; 1.0234x over previous
"""BezierAlign Trainium2 kernel.

Full inputs -> full output. Shards the R=256 ROIs across 8 NeuronCores (32
ROIs/core); the feature map is replicated to every core in a "window block"
layout: block(n,y,x) holds the 4-wide x 2-tall fp16 pixel window
[f(y,x..x+3) x f(y..y+1)] = 8C values = 4KB, so ONE indirect-DMA descriptor
fetches the footprint of BOTH x-samples of a bin row (max bin width 2.5 px
=> x_low spread <= 2). Indirect gathers are descriptor-rate-bound (~8.6 ns
per descriptor), so descriptor count is what matters.

Partition layout packs the TWO y-sample rows into the matmul contraction:
p = iy*64 + w (iy = y-sample row, w = output column). A tile is one output
row (64 bins). One gather per tile fetches 128 rows = both y-samples of all
64 bins; one matmul per (window-block, C-half) contracts both y-samples
via a stacked double-diagonal rhs [128, 64], so the PE streams only 64
columns per matmul (16 col/bin total vs 32 for the naive scheme) and the
diag-build DVE work is halved.

Per-core device program:
  1. Evaluate the 4 cubic Bezier curves per ROI on 32 partitions (roi-major),
     fold the +-0.25*bin sample offsets and the -0.5 align shift into shifted
     endpoint curves, PE-transpose them (x-curves duplicated across halves,
     y-curves iy-stacked) to the p = (iy, w) layout.
  2. Per ROI, compute sample coords / validity / per-window weights W8
     (4 x-positions x 2 y-rows, with dx-selection masks merging the two
     x-samples) / gather offsets, all on (iy, w) partitions, f32 DVE.
  3. Per 64-bin tile: 1 indirect gather ([128, 8C] fp16), 8 double-diag
     builds, 16 fp16 matmuls accumulating [C-half, 64 bins] in 2 PSUM
     tiles; copy to SBUF, DMA out.
"""

import numpy as np

# problem shapes (hardcoded per contract)
N, C, H, W = 2, 256, 160, 160
R = 256
OUT_H, OUT_W = 16, 64
SCALE = 0.25
NCORES = 8
K = R // NCORES          # 32 rois per core
NT = OUT_H               # 16 tiles of 64 bins (one output row) per roi
HW = H * W
NB = 8                   # blocks per window: 4 dx * 2 y

_CACHE = {}


def _host_constants():
    f32 = np.float32
    u = (np.arange(OUT_W, dtype=f32) / f32(OUT_W)).astype(f32)
    mt = (f32(1.0) - u).astype(f32)
    basis = np.stack([mt**3, 3 * u * mt**2, 3 * u**2 * mt, u**3]).astype(f32)  # [4,64]
    basis32 = np.broadcast_to(basis.reshape(1, 4 * OUT_W), (K, 4 * OUT_W)).copy()
    t = np.arange(NT, dtype=f32) / f32(NT)
    v16 = np.broadcast_to(t[None, :], (128, NT)).copy()      # [128,16] row v
    return basis32, v16


def _build_feat8(x):
    """x [N, C, H, W] f32 -> fp16 [N*H*W, 8C]; block(n,y,x) =
    [f(y,x), f(y+1,x), f(y,x+1), f(y+1,x+1), ..., f(y,x+3), f(y+1,x+3)]
    with out-of-image parts zeroed."""
    f = np.ascontiguousarray(x.transpose(0, 2, 3, 1)).astype(np.float16)  # [N,H,W,C]
    fy = np.zeros_like(f)
    fy[:, :-1] = f[:, 1:]
    a = np.concatenate([f, fy], axis=-1)                  # [N,H,W,2C] y-pair
    parts = [a]
    for dx in (1, 2, 3):
        ax = np.zeros_like(a)
        ax[:, :, :-dx] = a[:, :, dx:]
        parts.append(ax)
    feat8 = np.concatenate(parts, axis=-1)                # [N,H,W,8C]
    return np.ascontiguousarray(feat8.reshape(N * HW, NB * C))


def _build_nc(nrep=1):
    from contextlib import ExitStack
    import concourse.bacc as bacc
    import concourse.bass as bass
    import concourse.tile as tile
    from concourse import mybir
    from concourse.masks import make_identity

    f32 = mybir.dt.float32
    f16 = mybir.dt.float16
    i32 = mybir.dt.int32
    Alu = mybir.AluOpType

    nc = bacc.Bacc(None, target_bir_lowering=False)

    feat8 = nc.dram_tensor("feat8", [N * HW, NB * C], f16, kind="ExternalInput")
    rois = nc.dram_tensor("rois", [K, 17], f32, kind="ExternalInput")
    basis = nc.dram_tensor("basis", [K, 4 * OUT_W], f32, kind="ExternalInput")
    v16c = nc.dram_tensor("v16c", [128, NT], f32, kind="ExternalInput")
    out = nc.dram_tensor("out", [K, C, OUT_H, OUT_W], f32, kind="ExternalOutput")
    # [K, C, 16, 64] -> (k, p, h, t, w): c = h*128 + p
    out_v = out.rearrange("k (h p) t w -> k p h t w", h=2)

    with tile.TileContext(nc) as tc, ExitStack() as ctx:
        singles = ctx.enter_context(tc.tile_pool(name="singles", bufs=1))
        scratch = ctx.enter_context(tc.tile_pool(name="scratch", bufs=2))
        tabs = ctx.enter_context(tc.tile_pool(name="tabs", bufs=4))
        gpool = ctx.enter_context(tc.tile_pool(name="gpool", bufs=8))
        dpool = ctx.enter_context(tc.tile_pool(name="dpool", bufs=24))
        spool = ctx.enter_context(tc.tile_pool(name="spool", bufs=6))
        pp_t = ctx.enter_context(tc.tile_pool(name="pp_t", bufs=1, space="PSUM"))
        pp_mm = ctx.enter_context(tc.tile_pool(name="pp_mm", bufs=3, space="PSUM"))

        ident = singles.tile([128, 128], f32)
        make_identity(nc, ident[:])
        ident_h = singles.tile([128, 128], f16)
        nc.vector.tensor_copy(out=ident_h[:], in_=ident[:])
        # ID2 [128, 64] fp16: stacked pair of 64-identities (rows 0-63 and
        # 64-127 both diag on columns 0-63)
        ID2 = singles.tile([128, 64], f16)
        nc.vector.tensor_copy(out=ID2[0:64, :], in_=ident_h[0:64, 0:64])
        nc.vector.tensor_copy(out=ID2[64:128, :], in_=ident_h[64:128, 64:128])
        v16_t = singles.tile([128, NT], f32)
        nc.sync.dma_start(out=v16_t[:], in_=v16c[:])
        r_t = singles.tile([K, 17], f32)
        nc.sync.dma_start(out=r_t[:], in_=rois[:])
        b_t = singles.tile([K, 4, OUT_W], f32)
        nc.sync.dma_start(out=b_t[:], in_=basis[:].rearrange("k (a u) -> k a u", a=4))

        # control points: px = rois[:, 1::2]*0.25, py = rois[:, 2::2]*0.25
        px = scratch.tile([K, 8], f32, tag="px")
        py = scratch.tile([K, 8], f32, tag="py")
        r_ap = r_t[:]
        px_src = bass.AP(tensor=r_ap.tensor, offset=r_ap.offset + 1, ap=[list(r_ap.ap[0]), [2, 8]])
        py_src = bass.AP(tensor=r_ap.tensor, offset=r_ap.offset + 2, ap=[list(r_ap.ap[0]), [2, 8]])
        nc.vector.tensor_scalar(out=px[:], in0=px_src, scalar1=SCALE, scalar2=None, op0=Alu.mult)
        nc.vector.tensor_scalar(out=py[:], in0=py_src, scalar1=SCALE, scalar2=None, op0=Alu.mult)

        # curves [K, 64]: cv = sum_a B[a] * p[a(+4)]
        def bezier(dst, ptile, o):
            acc = scratch.tile([K, OUT_W], f32, tag="bzacc")
            tmp = scratch.tile([K, OUT_W], f32, tag="bztmp")
            nc.vector.tensor_scalar(out=acc[:], in0=b_t[:, 0, :], scalar1=ptile[:, o:o+1],
                                    scalar2=None, op0=Alu.mult)
            for a in (1, 2, 3):
                nc.vector.tensor_scalar(out=tmp[:], in0=b_t[:, a, :], scalar1=ptile[:, o+a:o+a+1],
                                        scalar2=None, op0=Alu.mult)
                nc.vector.tensor_tensor(out=dst[:] if a == 3 else acc[:],
                                        in0=acc[:], in1=tmp[:], op=Alu.add)

        x0 = scratch.tile([K, OUT_W], f32, tag="x0"); bezier(x0, px, 0)
        x1 = scratch.tile([K, OUT_W], f32, tag="x1"); bezier(x1, px, 4)
        y0 = scratch.tile([K, OUT_W], f32, tag="y0"); bezier(y0, py, 0)
        y1 = scratch.tile([K, OUT_W], f32, tag="y1"); bezier(y1, py, 4)

        # roi_w/h -> bwq = roi_w*0.25/64, bhq = roi_h*0.25/16  [K,1]
        def quarter_bin(ptile, scale_imm, tag):
            d1 = scratch.tile([K, 1], f32, tag=tag + "d1")
            d2 = scratch.tile([K, 1], f32, tag=tag + "d2")
            dn = scratch.tile([K, 1], f32, tag=tag + "dn")
            q = scratch.tile([K, 1], f32, tag=tag)
            nc.vector.tensor_tensor(out=d1[:], in0=ptile[:, 0:1], in1=ptile[:, 3:4], op=Alu.subtract)
            nc.vector.tensor_scalar(out=dn[:], in0=d1[:], scalar1=-1.0, scalar2=None, op0=Alu.mult)
            nc.vector.tensor_tensor(out=d1[:], in0=d1[:], in1=dn[:], op=Alu.max)
            nc.vector.tensor_tensor(out=d2[:], in0=ptile[:, 4:5], in1=ptile[:, 7:8], op=Alu.subtract)
            nc.vector.tensor_scalar(out=dn[:], in0=d2[:], scalar1=-1.0, scalar2=None, op0=Alu.mult)
            nc.vector.tensor_tensor(out=d2[:], in0=d2[:], in1=dn[:], op=Alu.max)
            nc.vector.tensor_tensor(out=d1[:], in0=d1[:], in1=d2[:], op=Alu.max)
            nc.vector.tensor_scalar(out=q[:], in0=d1[:], scalar1=scale_imm, scalar2=None, op0=Alu.mult)
            return q

        bwq = quarter_bin(px, 0.25 / OUT_W, "bwq")
        bhq = quarter_bin(py, 0.25 / OUT_H, "bhq")

        # 8 shifted curves [K, 64]: order xm0 xm1 xp0 xp1 ym0 ym1 yp0 yp1
        curves = scratch.tile([K, 9, OUT_W], f32, tag="curves")
        spec = [(x0, bwq, Alu.subtract, 0), (x1, bwq, Alu.subtract, 1),
                (x0, bwq, Alu.add, 2), (x1, bwq, Alu.add, 3),
                (y0, bhq, Alu.subtract, 4), (y1, bhq, Alu.subtract, 5),
                (y0, bhq, Alu.add, 6), (y1, bhq, Alu.add, 7)]
        for cv, qq, op, idx in spec:
            nc.vector.tensor_scalar(out=curves[:, idx, :], in0=cv[:], scalar1=qq[:, 0:1],
                                    scalar2=0.5, op0=op, op1=Alu.subtract)
        # base = batch * HW broadcast along 64
        base_c = scratch.tile([K, 1], f32, tag="base_c")
        nc.vector.tensor_scalar(out=base_c[:], in0=r_t[:, 0:1], scalar1=float(HW),
                                scalar2=None, op0=Alu.mult)
        bc_ap = base_c[:]
        nc.vector.tensor_scalar(
            out=curves[:, 8, :],
            in0=bass.AP(tensor=bc_ap.tensor, offset=bc_ap.offset, ap=[list(bc_ap.ap[0]), [0, OUT_W]]),
            scalar1=0.0, scalar2=None, op0=Alu.add)

        # transpose to TT [128, 7, K], p = iy*64 + w:
        #  slots 0-3: x endpoint curves xm0 xp0 xm1 xp1, duplicated across
        #             iy halves (x is iy-independent)
        #  slot 4: Y0 = [ym0 | yp0] iy-stacked; slot 5: Y1 = [ym1 | yp1]
        #  slot 6: base, duplicated
        TT = singles.tile([128, 7, K], f32)
        tt_spec = [(0, (0, 0)), (1, (1, 1)), (2, (2, 2)), (3, (3, 3)),
                   (4, (4, 6)), (5, (5, 7)), (6, (8, 8))]
        for slot, (qlo, qhi) in tt_spec:
            ps = pp_t.tile([128, K], f32, tag="tps", space="PSUM")
            cdup = scratch.tile([K, 128], f32, tag="cdup")
            nc.vector.tensor_copy(out=cdup[:, 0:64], in_=curves[:, qlo, :])
            nc.vector.tensor_copy(out=cdup[:, 64:128], in_=curves[:, qhi, :])
            nc.tensor.transpose(out=ps[:], in_=cdup[:], identity=ident[:K, :K])
            nc.vector.tensor_copy(out=TT[:, slot, :], in_=ps[:])

        def ttcol(q, r):
            return TT[:, q, r:r+1]

        IX, T16 = 2, NT

        def do_pipe(r):
            # deltas [128,1]: x-lerp slopes per ix, y-lerp slope (iy-stacked)
            dxm = tabs.tile([128, 1], f32, tag="dxm")
            dxp = tabs.tile([128, 1], f32, tag="dxp")
            dyy = tabs.tile([128, 1], f32, tag="dyy")
            nc.vector.tensor_tensor(out=dxm[:], in0=ttcol(1, r), in1=ttcol(0, r), op=Alu.subtract)
            nc.vector.tensor_tensor(out=dxp[:], in0=ttcol(3, r), in1=ttcol(2, r), op=Alu.subtract)
            nc.vector.tensor_tensor(out=dyy[:], in0=ttcol(5, r), in1=ttcol(4, r), op=Alu.subtract)

            # XX [128, 2(ix), 16(t)]; YY [128, 16(t)] (iy via partition half)
            XX = tabs.tile([128, IX, T16], f32, tag="XX")
            YY = tabs.tile([128, T16], f32, tag="YY")
            nc.vector.tensor_scalar(out=XX[:, 0, :], in0=v16_t[:], scalar1=dxm[:, 0:1],
                                    scalar2=ttcol(0, r), op0=Alu.mult, op1=Alu.add)
            nc.vector.tensor_scalar(out=XX[:, 1, :], in0=v16_t[:], scalar1=dxp[:, 0:1],
                                    scalar2=ttcol(2, r), op0=Alu.mult, op1=Alu.add)
            nc.vector.tensor_scalar(out=YY[:], in0=v16_t[:], scalar1=dyy[:, 0:1],
                                    scalar2=ttcol(4, r), op0=Alu.mult, op1=Alu.add)

            # coord pipe
            def pipe(P, F, limit, tagp):
                vv = tabs.tile([128, F], f32, tag=tagp + "v")
                v2 = tabs.tile([128, F], f32, tag=tagp + "v2")
                xx = tabs.tile([128, F], f32, tag=tagp + "x")
                xi = tabs.tile([128, F], i32, tag=tagp + "i")
                xf = tabs.tile([128, F], f32, tag=tagp + "f")
                xfc = tabs.tile([128, F], f32, tag=tagp + "fc")
                lo = tabs.tile([128, F], f32, tag=tagp + "lo")
                mm = tabs.tile([128, F], f32, tag=tagp + "m")
                lx = tabs.tile([128, F], f32, tag=tagp + "l")
                hx = tabs.tile([128, F], f32, tag=tagp + "h")
                nc.vector.tensor_scalar(out=vv[:], in0=P, scalar1=-1.0, scalar2=None, op0=Alu.is_gt)
                nc.vector.tensor_scalar(out=v2[:], in0=P, scalar1=float(limit), scalar2=None, op0=Alu.is_lt)
                nc.vector.tensor_tensor(out=vv[:], in0=vv[:], in1=v2[:], op=Alu.mult)
                nc.vector.tensor_scalar(out=xx[:], in0=P, scalar1=0.0, scalar2=None, op0=Alu.max)
                nc.vector.tensor_scalar(out=xi[:], in0=xx[:], scalar1=0.5, scalar2=None, op0=Alu.subtract)
                nc.vector.tensor_copy(out=xf[:], in_=xi[:])
                nc.vector.tensor_scalar(out=xfc[:], in0=xf[:], scalar1=float(limit - 1),
                                        scalar2=None, op0=Alu.min)
                nc.vector.tensor_tensor(out=lo[:], in0=xx[:], in1=xfc[:], op=Alu.subtract)
                nc.vector.tensor_scalar(out=mm[:], in0=xfc[:], scalar1=float(limit - 1),
                                        scalar2=None, op0=Alu.is_lt)
                nc.vector.tensor_tensor(out=lx[:], in0=lo[:], in1=mm[:], op=Alu.mult)
                nc.vector.tensor_scalar(out=hx[:], in0=lx[:], scalar1=-1.0, scalar2=1.0,
                                        op0=Alu.mult, op1=Alu.add)
                return vv, xfc, lx, hx

            vx, xfc, lx, hx = pipe(XX[:].rearrange("p a t -> p (a t)"), IX * T16, W, "px")
            vy, yfc, ly, hy = pipe(YY[:], T16, H, "py")

            # per-sample x validity fold: hxv = hx*vx, lxv = lx*vx [128, 32]
            hxv = tabs.tile([128, IX * T16], f32, tag="hxv")
            lxv = tabs.tile([128, IX * T16], f32, tag="lxv")
            nc.vector.tensor_tensor(out=hxv[:], in0=hx[:], in1=vx[:], op=Alu.mult)
            nc.vector.tensor_tensor(out=lxv[:], in0=lx[:], in1=vx[:], op=Alu.mult)

            # window-position indicators: d1 = xfc1 - xfc0 in {0,1,2} [128,16]
            d1 = tabs.tile([128, T16], f32, tag="d1")
            nc.vector.tensor_tensor(out=d1[:], in0=xfc[:, T16:2*T16], in1=xfc[:, 0:T16],
                                    op=Alu.subtract)
            i0 = tabs.tile([128, T16], f32, tag="i0")
            i1 = tabs.tile([128, T16], f32, tag="i1")
            i2 = tabs.tile([128, T16], f32, tag="i2")
            nc.vector.tensor_scalar(out=i0[:], in0=d1[:], scalar1=0.5, scalar2=None, op0=Alu.is_lt)
            nc.vector.tensor_scalar(out=i2[:], in0=d1[:], scalar1=1.5, scalar2=None, op0=Alu.is_gt)
            nc.vector.tensor_tensor(out=i1[:], in0=i0[:], in1=i2[:], op=Alu.add)
            nc.vector.tensor_scalar(out=i1[:], in0=i1[:], scalar1=-1.0, scalar2=1.0,
                                    op0=Alu.mult, op1=Alu.add)

            # x-weight planes WXT [128, 4(dx), 16(t)]
            # wx0 = hxv0 + i0*hxv1 ; wx1 = lxv0 + i0*lxv1 + i1*hxv1
            # wx2 = i1*lxv1 + i2*hxv1 ; wx3 = i2*lxv1
            WXT = tabs.tile([128, 4, T16], f32, tag="WXT")
            tmpa = tabs.tile([128, T16], f32, tag="tmpa")
            tmpb = tabs.tile([128, T16], f32, tag="tmpb")
            h1 = hxv[:, T16:2*T16]
            l1 = lxv[:, T16:2*T16]
            nc.vector.tensor_tensor(out=tmpa[:], in0=i0[:], in1=h1, op=Alu.mult)
            nc.vector.tensor_tensor(out=WXT[:, 0, :], in0=hxv[:, 0:T16], in1=tmpa[:], op=Alu.add)
            nc.vector.tensor_tensor(out=tmpa[:], in0=i0[:], in1=l1, op=Alu.mult)
            nc.vector.tensor_tensor(out=tmpb[:], in0=i1[:], in1=h1, op=Alu.mult)
            nc.vector.tensor_tensor(out=tmpa[:], in0=tmpa[:], in1=tmpb[:], op=Alu.add)
            nc.vector.tensor_tensor(out=WXT[:, 1, :], in0=lxv[:, 0:T16], in1=tmpa[:], op=Alu.add)
            nc.vector.tensor_tensor(out=tmpa[:], in0=i1[:], in1=l1, op=Alu.mult)
            nc.vector.tensor_tensor(out=tmpb[:], in0=i2[:], in1=h1, op=Alu.mult)
            nc.vector.tensor_tensor(out=WXT[:, 2, :], in0=tmpa[:], in1=tmpb[:], op=Alu.add)
            nc.vector.tensor_tensor(out=WXT[:, 3, :], in0=i2[:], in1=l1, op=Alu.mult)

            # offsets: o = (yfc*W + base) + xfc0  [128, 16] -> int32
            yw = tabs.tile([128, T16], f32, tag="yw")
            nc.vector.tensor_scalar(out=yw[:], in0=yfc[:], scalar1=float(W),
                                    scalar2=ttcol(6, r), op0=Alu.mult, op1=Alu.add)
            of = tabs.tile([128, T16], f32, tag="of")
            nc.vector.tensor_tensor(out=of[:], in0=yw[:], in1=xfc[:, 0:T16], op=Alu.add)
            O = tabs.tile([128, T16], i32, tag="O")
            nc.vector.tensor_copy(out=O[:], in_=of[:])

            # y weights folded with valid & 1/4: hyq = hy*vy*0.25, lyq likewise
            q0 = tabs.tile([128, T16], f32, tag="q0")
            nc.vector.tensor_scalar(out=q0[:], in0=vy[:], scalar1=0.25, scalar2=None, op0=Alu.mult)
            hyq = tabs.tile([128, T16], f32, tag="hyq")
            lyq = tabs.tile([128, T16], f32, tag="lyq")
            nc.vector.tensor_tensor(out=hyq[:], in0=hy[:], in1=q0[:], op=Alu.mult)
            nc.vector.tensor_tensor(out=lyq[:], in0=ly[:], in1=q0[:], op=Alu.mult)

            # W8 [128, 8 (2dx+ylh), 16 (t)]: wx[dx] x (hyq, lyq)
            W8 = tabs.tile([128, NB, T16], f32, tag="W8")
            for dx in range(4):
                nc.vector.tensor_tensor(out=W8[:, 2*dx, :], in0=hyq[:], in1=WXT[:, dx, :], op=Alu.mult)
                nc.vector.tensor_tensor(out=W8[:, 2*dx+1, :], in0=lyq[:], in1=WXT[:, dx, :], op=Alu.mult)
            return O, W8

        def do_tiles(r, O, W8):
            # per tile (one output row, 64 bins): gather + combine into
            # [C-half, 64] PSUM via double-diag matmuls
            st = None
            for t in range(T16):
                g = gpool.tile([128, NB * C], f16, tag="g")
                nc.gpsimd.indirect_dma_start(
                    out=g[:], out_offset=None, in_=feat8[:],
                    in_offset=bass.IndirectOffsetOnAxis(ap=O[:, t:t+1], axis=0))
                # double-diags for this tile, fused in PAIRS (128-el DVE ops;
                # wider fusions slow the indirect gathers via SBUF contention):
                # DG8[p, blk, b] = ID2[p, b] * W8[p, blk, t]
                DG8 = dpool.tile([128, NB, 64], f16, tag="DG8")
                id_ap = ID2[:]
                id_b = bass.AP(tensor=id_ap.tensor, offset=id_ap.offset,
                               ap=[list(id_ap.ap[0]), [0, 2], [1, 64]])
                w8_ap = W8[:]
                for bp in range(NB // 2):
                    w8_b = bass.AP(tensor=w8_ap.tensor,
                                   offset=w8_ap.offset + (2 * bp) * T16 + t,
                                   ap=[list(w8_ap.ap[0]), [T16, 2], [0, 64]])
                    nc.vector.tensor_tensor(out=DG8[:, 2*bp:2*bp+2, :],
                                            in0=id_b, in1=w8_b, op=Alu.mult)
                psA = pp_mm.tile([128, 64], f32, tag="psA", space="PSUM")
                psB = pp_mm.tile([128, 64], f32, tag="psB", space="PSUM")
                for blk in range(NB):
                    nc.tensor.matmul(psA[:], lhsT=g[:, blk*C:blk*C+128],
                                     rhs=DG8[:, blk, :],
                                     start=(blk == 0), stop=(blk == NB - 1))
                    nc.tensor.matmul(psB[:], lhsT=g[:, blk*C+128:blk*C+256],
                                     rhs=DG8[:, blk, :],
                                     start=(blk == 0), stop=(blk == NB - 1))
                # stage 2 tiles per out-DMA (512B runs per partition)
                if t % 2 == 0:
                    st = spool.tile([128, 2, 2, 64], f32, tag="st")
                nc.scalar.copy(st[:, 0, t % 2, :], psA[:])
                nc.scalar.copy(st[:, 1, t % 2, :], psB[:])
                if t % 2 == 1:
                    nc.sync.dma_start(out=out_v[r, :, :, t-1:t+1, :], in_=st[:])

        def main_work():
            # software-pipeline: emit roi r+1's coordinate pipe BEFORE roi
            # r's tile crunch so O/W8 are ready when the gathers drain
            pend = do_pipe(0)
            for r in range(K):
                cur = pend
                if r + 1 < K:
                    pend = do_pipe(r + 1)
                do_tiles(r, *cur)

        if nrep > 1:
            with tc.For_i(0, nrep, 1):
                main_work()
        else:
            main_work()

    nc.finalize()
    return nc


def _get_nc():
    if "nc" not in _CACHE:
        _CACHE["nc"] = _build_nc()
    return _CACHE["nc"]


def run_sharded(input, rois, **spmd_kwargs):
    """Run on 8 cores; returns (full_output, BassKernelResults)."""
    from concourse.bass_utils import run_bass_kernel_spmd

    x = np.ascontiguousarray(np.asarray(input, dtype=np.float32))
    rr = np.ascontiguousarray(np.asarray(rois, dtype=np.float32))
    feat8 = _build_feat8(x)
    basis32, v16 = _host_constants()

    in_maps = []
    for c in range(NCORES):
        in_maps.append({
            "feat8": feat8,
            "rois": np.ascontiguousarray(rr[c*K:(c+1)*K]),
            "basis": basis32,
            "v16c": v16,
        })
    nc = _get_nc()
    res = run_bass_kernel_spmd(nc, in_maps, core_ids=list(range(NCORES)), **spmd_kwargs)
    outp = np.concatenate([res.results[c]["out"] for c in range(NCORES)], axis=0)
    return outp, res


def kernel(input, rois):
    out, _ = run_sharded(input, rois)
    return out
